# revision 1
# baseline (speedup 1.0000x reference)
"""GCN 2-layer encoder on 8 TRN2 NeuronCores (Bass/Tile).

Math (PyG GCNConv, symmetric normalization, self-loops, deg from dst):
    out1 = relu(Dh @ A @ Dh @ (x @ W1) + b1),  Dh = diag(deg^-1/2)
    out  = Dh @ A @ Dh @ (relu1 @ W2) + b2

Factorization used here (per layer):
    table = Dh @ (feat @ W)          # per-node rows, built on device
    agg[d] = sum_{e: src->d} table[src]   (self loops included as edges)
    out[d] = dinv[d] * agg[d] + b

Sharding: nodes are assigned to 8 cores (balanced by in-degree); each core
aggregates its own dst nodes. Per dst tile (128 nodes), in-edges are packed
densely into chunks of 128 lanes; gathered message chunks [128 lanes, F]
are multiplied on the PE by a per-chunk multi-hot sigma (lane -> dst col)
accumulating in PSUM. Sigma matrices are built on-device by the Vector
engine (iota == colidx compare) from compact per-lane column indices, so
lanes need no static lane->node binding and padding is just the final
partial chunk per (tile, side): ~5% vs ~56% for per-tile sigma.

Messages are fetched with SWDGE dma_gather in prepare_only mode + explicit
trigger_dma: desc-gen (~0.7us/call) is decoupled from the DMA transfer
(~5-8us/call), which otherwise blocks the GpSimd engine. Post-compile
surgery (_fix_swdge_prep_sems) wires the descriptor completion sems to the
DMASW lane sems Tile's consumers actually wait on, throttles to one
in-flight call per lane, and restores the dropped write-after-read hazard
(trigger vs. previous staging-slot readers).

Since gather indices are int16, the node table is split in two blocks
(cores 0-3 / cores 4-7); each (tile-group, block) run is a separate call.

Layer-1 tables are built replicated on every core; the layer-2 table is
built sharded and exchanged with one AllGather.
"""

import sys
import types

sys.path.insert(0, "/opt/trn_rl_repo")

import numpy as np

# Register the NTFF profile hook the container's antenv stub lacks, so
# BASS_TRACE=1 profiling works under axon (harmless otherwise).
if "antenv.axon_hooks" not in sys.modules:
    try:
        from trn_agent_boot.trn_boot import _ntff_profile_via_ctypes

        _hook = _ntff_profile_via_ctypes("/opt/axon/libaxon_pjrt.so")
    except Exception:
        _hook = None
    _m = types.ModuleType("antenv.axon_hooks")
    _m.get_axon_ntff_profile_hook = lambda: _hook
    sys.modules["antenv.axon_hooks"] = _m

N = 50000
E = 800000
IN_CH = 128
HID = 128
OUT_CH = 64
NCORES = 8
P = 128
GSZ = 4  # tiles per gather call group
CALL_CAP = 8  # max chunks (x128 idxs) per dma_gather call (16KB/engine packet)
SWDGE_QUEUES = 4
BB = 4  # phase-1 DMA batching (tiles per dma_start)

_CACHE = {}
LAST_RESULTS = None


# ----------------------------------------------------------------------------
# Host-side planning
# ----------------------------------------------------------------------------
def _plan(edge_index):
    src = np.asarray(edge_index[0], dtype=np.int64)
    dst = np.asarray(edge_index[1], dtype=np.int64)
    loops = np.arange(N, dtype=np.int64)
    src_all = np.concatenate([src, loops])
    dst_all = np.concatenate([dst, loops])
    deg = np.bincount(dst_all, minlength=N)
    dinv = (1.0 / np.sqrt(deg.astype(np.float64))).astype(np.float32)

    # node -> core: snake over degree-sorted nodes (balances sum(deg))
    order = np.argsort(-deg, kind="stable")
    snake = np.tile(
        np.concatenate([np.arange(NCORES), np.arange(NCORES - 1, -1, -1)]),
        N // (2 * NCORES) + 1,
    )[:N]
    core_of = np.empty(N, dtype=np.int64)
    core_of[order] = snake

    isA = core_of[src_all] < (NCORES // 2)
    a_cnt = np.bincount(dst_all[isA], minlength=N)
    b_cnt = np.bincount(dst_all[~isA], minlength=N)

    # node -> (tile, col): per core, snake over degree-sorted nodes across
    # provisional tiles (balances per-tile edge sums), tiles then sorted by
    # chunk need desc (aligns profiles across cores) and renumbered.
    tile_of = np.full(N, -1, dtype=np.int64)
    col_of = np.full(N, -1, dtype=np.int64)
    ntiles_max = 0
    prov = []
    for c in range(NCORES):
        nodes = np.where(core_of == c)[0]
        nn = len(nodes)
        ntiles = -(-nn // P)
        ntiles_max = max(ntiles_max, ntiles)
        o2 = np.argsort(-(a_cnt[nodes] + b_cnt[nodes]), kind="stable")
        nds = nodes[o2]
        sn = np.tile(
            np.concatenate([np.arange(ntiles), np.arange(ntiles - 1, -1, -1)]),
            nn // (2 * ntiles) + 1,
        )[:nn]
        prov.append([nds[sn == t] for t in range(ntiles)])

    T = ntiles_max
    ca_t = np.zeros((NCORES, T), dtype=np.int64)
    cb_t = np.zeros((NCORES, T), dtype=np.int64)
    for c in range(NCORES):
        for t, nds in enumerate(prov[c]):
            ca_t[c, t] = -(-int(a_cnt[nds].sum()) // P)
            cb_t[c, t] = -(-int(b_cnt[nds].sum()) // P)
    CA = np.zeros(T, dtype=np.int64)
    CB = np.zeros(T, dtype=np.int64)
    for c in range(NCORES):
        perm = sorted(
            range(len(prov[c])), key=lambda t: -(ca_t[c, t] + cb_t[c, t])
        )
        for p_, t in enumerate(perm):
            nds = prov[c][t]
            tile_of[nds] = p_
            col_of[nds] = np.arange(len(nds))
            CA[p_] = max(CA[p_], ca_t[c, t])
            CB[p_] = max(CB[p_], cb_t[c, t])
    CA[(CA + CB) == 0] = 1

    SLOTS = T * P
    HALFROWS = (NCORES // 2) * SLOTS
    assert HALFROWS <= 32768, HALFROWS
    slot_of = tile_of * P + col_of
    pos_of = core_of * SLOTS + slot_of

    # pad rows: any unoccupied slot is a zero row in both tables (zero x,
    # dinv=0). Find one in core 3 (A half) and core 7 (B half).
    def free_slot(c):
        occ = np.zeros(SLOTS, dtype=bool)
        occ[slot_of[core_of == c]] = True
        fr = np.where(~occ)[0]
        assert len(fr) > 0
        return int(fr[-1])

    PAD_A = (NCORES // 2 - 1) * SLOTS + free_slot(NCORES // 2 - 1)
    PAD_B = (NCORES // 2 - 1) * SLOTS + free_slot(NCORES - 1)

    ecore = core_of[dst_all]
    etile = tile_of[dst_all]
    eside = (~isA).astype(np.int64)
    esrcpos = pos_of[src_all]
    ecol = col_of[dst_all]

    G = -(-T // GSZ)
    groups = [list(range(g * GSZ, min((g + 1) * GSZ, T))) for g in range(G)]
    tot_chunks = int(np.sum(CA) + np.sum(CB))
    maxc_call = 0
    for g in groups:
        ca_g = int(sum(CA[p_] for p_ in g))
        cb_g = int(sum(CB[p_] for p_ in g))
        maxc_call = max(maxc_call, ca_g, cb_g)

    ekey = np.lexsort((esrcpos, etile, eside, ecore))
    es_core = ecore[ekey]
    es_side = eside[ekey]
    es_tile = etile[ekey]
    es_srcpos = esrcpos[ekey]
    es_col = ecol[ekey]
    keyv = (es_core * 2 + es_side) * T + es_tile
    uniq, starts = np.unique(keyv, return_index=True)
    ends = np.append(starts[1:], len(keyv))
    bnd = {int(u): (int(s0), int(e0)) for u, s0, e0 in zip(uniq, starts, ends)}

    idx_cores = []
    colidx_cores = []
    dinv_own_cores = []
    for c in range(NCORES):
        flat_idx = []
        flat_col = []
        dvo = np.zeros((P, T), dtype=np.float32)
        nds_c = np.where(core_of == c)[0]
        dvo[col_of[nds_c], tile_of[nds_c]] = dinv[nds_c]

        def emit(side, t, nchunks):
            k = (c * 2 + side) * T + t
            s0, e0 = bnd.get(k, (0, 0))
            sp = es_srcpos[s0:e0]
            cl = es_col[s0:e0]
            if side == 1:
                sp = sp - HALFROWS
            n_ = e0 - s0
            want = nchunks * P
            ii = np.full(want, PAD_A if side == 0 else PAD_B, np.int64)
            cc = np.full(want, P - 1, np.int64)
            ii[:n_] = sp
            cc[:n_] = cl
            flat_idx.append(ii)
            flat_col.append(cc)

        for g in groups:
            for p_ in g:
                emit(0, p_, int(CA[p_]))
            for p_ in g:
                emit(1, p_, int(CB[p_]))
        fi = np.concatenate(flat_idx)
        fc = np.concatenate(flat_col)
        assert fi.size == tot_chunks * P
        assert fi.min() >= 0 and fi.max() < HALFROWS
        wrapped = fi.astype(np.int16).reshape(-1, 16).T.copy()
        idx_cores.append(np.tile(wrapped, (8, 1)))
        colidx_cores.append(fc.reshape(tot_chunks, P).T.astype(np.float16).copy())
        dinv_own_cores.append(dvo)

    dinv_all = np.zeros((P, NCORES * T), dtype=np.float32)
    for c in range(NCORES):
        dinv_all[:, c * T : (c + 1) * T] = dinv_own_cores[c]

    iotaC = np.tile(
        np.arange(P, dtype=np.float16)[None, :], (P, maxc_call)
    ).reshape(P, maxc_call * P)

    return dict(
        T=T,
        SLOTS=SLOTS,
        CA=CA,
        CB=CB,
        groups=groups,
        tot_chunks=tot_chunks,
        maxc_call=maxc_call,
        core_of=core_of,
        slot_of=slot_of,
        pos_of=pos_of,
        dinv=dinv,
        idx_cores=idx_cores,
        colidx_cores=colidx_cores,
        dinv_own_cores=dinv_own_cores,
        dinv_all=dinv_all,
        iotaC=iotaC,
    )


# ----------------------------------------------------------------------------
# Device kernel
# ----------------------------------------------------------------------------
def _build(
    T,
    CA,
    CB,
    groups,
    tot_chunks,
    maxc_call,
    use_collective=True,
    detect_races=True,
):
    import concourse.bass as bass
    import concourse.mybir as mybir
    import concourse.tile as tile
    from concourse import bacc

    f16 = mybir.dt.float16
    f32 = mybir.dt.float32
    i16 = mybir.dt.int16
    SLOTS = T * P
    ROWS = NCORES * SLOTS
    HALFROWS = ROWS // 2
    NT = NCORES * T

    nc = bacc.Bacc(
        "TRN2",
        target_bir_lowering=False,
        num_devices=NCORES,
        num_swdge_queues=SWDGE_QUEUES,
        detect_race_conditions=detect_races,
    )
    qn = [0]

    def _next_q():
        qn[0] = (qn[0] + 1) % SWDGE_QUEUES
        return qn[0]

    # R rotating completion sems per queue: consumer waits target call's
    # rotation sem, so a premature unblock needs >= R calls of inter-engine
    # skew on one queue instead of 1.
    dma_sems = [
        [nc.alloc_semaphore(f"swdge_dma_q{i}r{r}") for r in range(SEM_ROT)]
        for i in range(SWDGE_QUEUES)
    ]
    q_calls = [0] * SWDGE_QUEUES

    def _prep(out_ap, in_ap, idx_ap, n_idx, q):
        jq = q_calls[q]
        q_calls[q] += 1
        nc.gpsimd.dma_gather(
            out_ap,
            in_ap,
            idx_ap,
            n_idx,
            n_idx,
            P,
            prepare_only=True,
            sem=dma_sems[q][jq % SEM_ROT],
            queue_num=q,
        )

    def _fire(q):
        # One trigger per (group, side): the trigger blocks the GpSimd
        # engine ~9us on HW regardless of how many calls it fires, so
        # batch all of a side's calls onto one queue and fire once.
        nc.gpsimd.trigger_dma(count=None, queue_num=q)

    xT_in = nc.dram_tensor("xT", [NT, P, P], f16, kind="ExternalInput")
    w1_in = nc.dram_tensor("W1", [IN_CH, HID], f16, kind="ExternalInput")
    w2_in = nc.dram_tensor("W2", [HID, OUT_CH], f16, kind="ExternalInput")
    b1_in = nc.dram_tensor("b1bc", [P, HID], f32, kind="ExternalInput")
    b2_in = nc.dram_tensor("b2bc", [P, OUT_CH], f32, kind="ExternalInput")
    id_in = nc.dram_tensor("ident", [P, P], f16, kind="ExternalInput")
    col_in = nc.dram_tensor("colidx", [P, tot_chunks], f16, kind="ExternalInput")
    iota_in = nc.dram_tensor("iotaC", [P, maxc_call * P], f16, kind="ExternalInput")
    da_in = nc.dram_tensor("dinv_all", [P, NT], f32, kind="ExternalInput")
    do_in = nc.dram_tensor("dinv_own", [P, T], f32, kind="ExternalInput")
    idx_in = nc.dram_tensor("idx", [P, tot_chunks * 8], i16, kind="ExternalInput")
    out_ext = nc.dram_tensor("out", [SLOTS, OUT_CH], f32, kind="ExternalOutput")

    with tile.TileContext(nc) as tc:
        with (
            tc.tile_pool(name="const", bufs=1) as cpool,
            tc.tile_pool(name="xt", bufs=3) as xtpool,
            tc.tile_pool(name="sig", bufs=3) as sigpool,
            tc.tile_pool(name="stg", bufs=3) as stgpool,
            tc.tile_pool(name="drain", bufs=3) as dpool,
            tc.tile_pool(name="psb", bufs=2, space="PSUM") as ps_build,
            tc.tile_pool(name="psa", bufs=3, space="PSUM") as ps_agg,
            tc.tile_pool(name="pst", bufs=2, space="PSUM") as ps_tr,
            tc.tile_pool(name="psm", bufs=1, space="PSUM") as ps_mm2,
            tc.tile_pool(name="dram", bufs=1, space="DRAM") as dram,
        ):
            # ---- constants into SBUF ----
            w1_sb = cpool.tile([IN_CH, HID], f16)
            nc.sync.dma_start(out=w1_sb[:], in_=w1_in[:])
            w2_sb = cpool.tile([HID, OUT_CH], f16)
            nc.sync.dma_start(out=w2_sb[:], in_=w2_in[:])
            b1_sb = cpool.tile([P, HID], f32)
            nc.sync.dma_start(out=b1_sb[:], in_=b1_in[:])
            b2_sb = cpool.tile([P, OUT_CH], f32)
            nc.sync.dma_start(out=b2_sb[:], in_=b2_in[:])
            id_sb = cpool.tile([P, P], f16)
            nc.sync.dma_start(out=id_sb[:], in_=id_in[:])
            col_sb = cpool.tile([P, tot_chunks], f16)
            nc.sync.dma_start(out=col_sb[:], in_=col_in[:])
            iota_sb = cpool.tile([P, maxc_call * P], f16)
            nc.sync.dma_start(out=iota_sb[:], in_=iota_in[:])
            da_sb = cpool.tile([P, NT], f32)
            nc.sync.dma_start(out=da_sb[:], in_=da_in[:])
            do_sb = cpool.tile([P, T], f32)
            nc.sync.dma_start(out=do_sb[:], in_=do_in[:])
            idx_sb = cpool.tile([P, tot_chunks * 8], i16)
            nc.sync.dma_start(out=idx_sb[:], in_=idx_in[:])

            table1 = dram.tile([ROWS, HID], f16)
            shard2 = dram.tile([SLOTS, P], f16)
            table2 = dram.tile(
                [ROWS, P], f16, addr_space="Shared" if use_collective else "Local"
            )

            # ---- phase 1: table1 = dinv * (x @ W1), full, replicated ----
            for j0 in range(0, NT, BB):
                nb = min(BB, NT - j0)
                xt_t = xtpool.tile([P, nb * P], f16, tag="xt")
                nc.sync.dma_start(
                    out=xt_t[:].rearrange("p (t c) -> p t c", t=nb),
                    in_=xT_in[j0 : j0 + nb].rearrange("t p c -> p t c"),
                )
                h1t = xtpool.tile([P, nb * HID], f16, tag="h1t")
                for k in range(nb):
                    j = j0 + k
                    bps = ps_build.tile([P, HID], f32, tag="build")
                    nc.tensor.matmul(
                        bps[:],
                        lhsT=xt_t[:, k * P : (k + 1) * P],
                        rhs=w1_sb[:],
                        start=True,
                        stop=True,
                    )
                    if j % 2 == 0:
                        nc.scalar.activation(
                            h1t[:, k * HID : (k + 1) * HID],
                            bps[:],
                            mybir.ActivationFunctionType.Copy,
                            scale=da_sb[:, j : j + 1],
                        )
                    else:
                        nc.vector.tensor_scalar_mul(
                            h1t[:, k * HID : (k + 1) * HID],
                            bps[:],
                            da_sb[:, j : j + 1],
                        )
                nc.sync.dma_start(
                    out=table1[j0 * P : (j0 + nb) * P, :].rearrange(
                        "(t p) f -> p t f", t=nb
                    ),
                    in_=h1t[:].rearrange("p (t f) -> p t f", t=nb),
                )

            # ---- per-layer aggregation ----
            def aggregate(layer):
                tab = table1 if layer == 0 else table2
                nfeat = HID if layer == 0 else OUT_CH
                coff = 0
                for g in groups:
                    ca_g = int(sum(int(CA[p_]) for p_ in g))
                    cb_g = int(sum(int(CB[p_]) for p_ in g))
                    stA = stB = sgA = sgB = None
                    if ca_g:
                        stA = stgpool.tile([P, maxc_call, P], f16, tag="stgA")
                        used = set()
                        for s_ in range(0, ca_g, CALL_CAP):
                            n_ = min(CALL_CAP, ca_g - s_)
                            q = _next_q()
                            used.add(q)
                            _prep(
                                stA[:, s_ : s_ + n_, :],
                                tab[0:HALFROWS, :],
                                idx_sb[:, (coff + s_) * 8 : (coff + s_ + n_) * 8],
                                n_ * P,
                                q,
                            )
                        for q in sorted(used):
                            _fire(q)
                        sgA = sigpool.tile([P, maxc_call * P], f16, tag="sgA")
                        nc.vector.tensor_tensor(
                            sgA[:, : ca_g * P].rearrange(
                                "p (k c) -> p k c", k=ca_g
                            ),
                            iota_sb[:, : ca_g * P].rearrange(
                                "p (k c) -> p k c", k=ca_g
                            ),
                            col_sb[:, coff : coff + ca_g]
                            .unsqueeze(-1)
                            .broadcast_to([P, ca_g, P]),
                            mybir.AluOpType.is_equal,
                        )
                    if cb_g:
                        stB = stgpool.tile([P, maxc_call, P], f16, tag="stgB")
                        used = set()
                        for s_ in range(0, cb_g, CALL_CAP):
                            n_ = min(CALL_CAP, cb_g - s_)
                            q = _next_q()
                            used.add(q)
                            _prep(
                                stB[:, s_ : s_ + n_, :],
                                tab[HALFROWS:ROWS, :],
                                idx_sb[
                                    :,
                                    (coff + ca_g + s_) * 8 : (coff + ca_g + s_ + n_)
                                    * 8,
                                ],
                                n_ * P,
                                q,
                            )
                        for q in sorted(used):
                            _fire(q)
                        sgB = sigpool.tile([P, maxc_call * P], f16, tag="sgB")
                        nc.vector.tensor_tensor(
                            sgB[:, : cb_g * P].rearrange(
                                "p (k c) -> p k c", k=cb_g
                            ),
                            iota_sb[:, : cb_g * P].rearrange(
                                "p (k c) -> p k c", k=cb_g
                            ),
                            col_sb[:, coff + ca_g : coff + ca_g + cb_g]
                            .unsqueeze(-1)
                            .broadcast_to([P, cb_g, P]),
                            mybir.AluOpType.is_equal,
                        )
                    a_off = 0
                    b_off = 0
                    for p_ in g:
                        aps = ps_agg.tile([P, nfeat], f32, tag="agg")
                        ntot = int(CA[p_]) + int(CB[p_])
                        k = 0
                        for ci in range(int(CA[p_])):
                            cc = a_off + ci
                            nc.tensor.matmul(
                                aps[:],
                                lhsT=sgA[:, cc * P : (cc + 1) * P],
                                rhs=stA[:, cc, 0:nfeat],
                                start=(k == 0),
                                stop=(k == ntot - 1),
                            )
                            k += 1
                        for ci in range(int(CB[p_])):
                            cc = b_off + ci
                            nc.tensor.matmul(
                                aps[:],
                                lhsT=sgB[:, cc * P : (cc + 1) * P],
                                rhs=stB[:, cc, 0:nfeat],
                                start=(k == 0),
                                stop=(k == ntot - 1),
                            )
                            k += 1
                        a_off += int(CA[p_])
                        b_off += int(CB[p_])
                        drain(layer, p_, aps)
                    coff += ca_g + cb_g

            def drain(layer, p_, aps):
                dv = do_sb[:, p_ : p_ + 1]
                if layer == 0:
                    # r1 = dinv*agg + b1 ; r3 = relu(r1)*dinv (fp16)
                    r1 = dpool.tile([P, HID], f32, tag="r1")
                    nc.scalar.activation(
                        r1[:], aps[:], mybir.ActivationFunctionType.Copy, scale=dv
                    )
                    nc.vector.tensor_add(r1[:], r1[:], b1_sb[:])
                    r3 = dpool.tile([P, HID], f16, tag="r3")
                    nc.vector.tensor_scalar(
                        r3[:], r1[:], 0.0, dv, mybir.AluOpType.max, mybir.AluOpType.mult
                    )
                    psT = ps_tr.tile([P, P], f16, tag="tr")
                    nc.tensor.transpose(psT[:], r3[:], id_sb[:])
                    rT = dpool.tile([P, P], f16, tag="rT")
                    nc.vector.tensor_copy(rT[:], psT[:])
                    ps2 = ps_mm2.tile([P, OUT_CH], f32, tag="mm2")
                    nc.tensor.matmul(
                        ps2[:], lhsT=rT[:], rhs=w2_sb[:], start=True, stop=True
                    )
                    t2 = dpool.tile([P, P], f16, tag="t2")
                    nc.scalar.activation(
                        t2[:, 0:OUT_CH], ps2[:], mybir.ActivationFunctionType.Copy
                    )
                    nc.vector.memset(t2[:, OUT_CH:P], 0.0)
                    nc.sync.dma_start(
                        out=shard2[p_ * P : (p_ + 1) * P, :], in_=t2[:]
                    )
                else:
                    o1 = dpool.tile([P, OUT_CH], f32, tag="o1")
                    nc.scalar.activation(
                        o1[:], aps[:], mybir.ActivationFunctionType.Copy, scale=dv
                    )
                    nc.vector.tensor_add(o1[:], o1[:], b2_sb[:])
                    nc.sync.dma_start(
                        out=out_ext[p_ * P : (p_ + 1) * P, :], in_=o1[:]
                    )

            aggregate(0)

            if use_collective:
                nc.gpsimd.collective_compute(
                    "AllGather",
                    mybir.AluOpType.bypass,
                    replica_groups=[list(range(NCORES))],
                    ins=[shard2.opt()],
                    outs=[table2.opt()],
                )
            else:
                for c_ in range(NCORES):
                    nc.sync.dma_start(
                        out=table2[c_ * SLOTS : (c_ + 1) * SLOTS, :], in_=shard2[:]
                    )

            aggregate(1)

    nc.compile()  # bacc passes: library loads, register allocation, DCE
    _fix_swdge_prep_sems(nc, mybir)
    _split_sync_waits(nc, mybir, max_waits=1)
    return nc


PREP_DEPTH = 10  # max in-flight gather calls per SWDGE queue
STG_BUFS = 3  # staging pool depth (groups in flight); must match tile_pool
SEM_ROT = 4  # rotating DMA-completion sems per queue


def _fix_swdge_prep_sems(nc, mybir):
    """Post-compile surgery for the gen_mode==1 SWDGE prep/trigger path.

    Tile treats prepare_only SWDGE completion as user-managed: it
    discharges the DMASW lane ticks with unconditional IncSwdgeSem
    pre-bumps, so the lane-sem waits it emits on consumers are vacuous.
    The author must enforce data readiness with the sem= semaphores
    (one per queue here, descriptors bump +16 per call). Enforce:

    1. Data RAW: the first matmul reading each staging-tile instance
       waits on every covering gather call: sem_q >= 16*(call# in q + 1).
    2. WAR: the trigger that fires a DMA overwriting a staging slot waits
       on PE engine sem >= (last matmul reading the slot's previous
       instance; with STG_BUFS pool bufs that is the same-tag instance
       STG_BUFS allocations back).
    3. Ring/throttle: prep #j on queue q waits sem_q >= 16*(j-D+1),
       capping in-flight calls per queue at D=PREP_DEPTH.
    """
    import re

    queue_sems = {}  # (q, r) -> (id, name)
    pe_sem = None
    for fn in nc.m.functions:
        for bb in fn.blocks:
            for ins in bb.instructions:
                si = ins.sync_info
                if not si:
                    continue
                for u in si.on_update or []:
                    nm = u.ant_name or ""
                    if nm.startswith("swdge_dma_q"):
                        qs, rs = nm[11:].split("r")
                        queue_sems[(int(qs), int(rs))] = (u.id, nm)
                for w in si.on_wait or []:
                    nm = w.ant_name or ""
                    if nm.startswith("PE_") and pe_sem is None:
                        pe_sem = (w.id, nm)

    pat = re.compile(r"\b(st[AB])_(\d+)\b")

    def _stg_name(ap):
        m = pat.search(str(ap))
        return m.group(0) if m else None

    streams = []
    for fn in nc.m.functions:
        for bb in fn.blocks:
            streams.append(bb.instructions)

    # pass 1: per-queue call indices per prep; staging instances: creation
    # order (per tag), covering calls, first/last matmul readers
    inst_order = {"stA": [], "stB": []}
    seen = set()
    first_reader = {}
    last_reader_n = {}
    inst_calls = {}
    prep_info = {}
    q_count = {}
    pe_n = 0
    for insts in streams:
        for ins in insts:
            tn = type(ins).__name__
            if tn == "InstMatmult":
                pe_n += 1
                for ap in ins.ins or []:
                    nm = _stg_name(ap)
                    if nm:
                        if nm not in first_reader:
                            first_reader[nm] = ins
                        last_reader_n[nm] = pe_n
            elif tn == "InstDMAGatherAnt" and getattr(ins, "gen_mode", 0) == 1:
                q = ins.queue_num
                jq = q_count.get(q, 0)
                q_count[q] = jq + 1
                nm = _stg_name(ins.outs[0])
                prep_info[ins.name] = (q, jq, nm)
                if nm:
                    inst_calls.setdefault(nm, []).append((q, jq))
                    if nm not in seen:
                        seen.add(nm)
                        inst_order[nm[:3]].append(nm)
    prev_inst = {}
    for tag, lst in inst_order.items():
        for i, nm in enumerate(lst):
            if i >= STG_BUFS:
                prev_inst[nm] = lst[i - STG_BUFS]

    def _add_wait(ins, sid, snm, val):
        si = ins.sync_info
        if si is None:
            si = mybir.SyncInfo(on_wait=[], on_update=[])
        si.on_wait = list(si.on_wait or []) + [
            mybir.SyncWait(
                sync_type="semaphore",
                id=sid,
                ant_name=snm,
                wait_mode="sem-ge-imm",
                wait_value=val,
                wait_reg=None,
            )
        ]
        ins.sync_info = si

    # 1. data RAW waits on first readers (per (queue, rotation) max target)
    for nm, rd in first_reader.items():
        per_qr = {}
        for q, jq in inst_calls.get(nm, []):
            k = (q, jq % SEM_ROT)
            per_qr[k] = max(per_qr.get(k, -1), jq // SEM_ROT)
        for k, t in sorted(per_qr.items()):
            if k in queue_sems:
                sid, snm = queue_sems[k]
                _add_wait(rd, sid, snm, 16 * (t + 1))

    # 2 + 3. WAR waits on triggers, throttle on preps; also gate the first
    # trigger after each collective on its completion (the gather source
    # table2 is written by the async AllGather).
    cc_sem = None
    for insts in streams:
        for ins in insts:
            si = ins.sync_info
            if not si:
                continue
            for w in si.on_wait or []:
                if (w.ant_name or "").startswith("Collectives"):
                    cc_sem = (w.id, w.ant_name)
    pending_prep = {}
    cc_count = 0
    cc_pending = set()
    for insts in streams:
        for ins in insts:
            tn = type(ins).__name__
            if tn == "InstCollectiveCompute":
                cc_count += 1
                cc_pending = set(range(SWDGE_QUEUES))
            elif tn == "InstDMAGatherAnt" and getattr(ins, "gen_mode", 0) == 1:
                q, jq, nm = prep_info[ins.name]
                pending_prep.setdefault(q, []).append(ins.name)
                if jq >= PREP_DEPTH:
                    jt = jq - PREP_DEPTH
                    k = (q, jt % SEM_ROT)
                    if k in queue_sems:
                        sid, snm = queue_sems[k]
                        _add_wait(ins, sid, snm, 16 * (jt // SEM_ROT + 1))
            elif tn == "InstTriggerDma":
                if ins.queue_num in cc_pending and cc_sem is not None:
                    _add_wait(ins, cc_sem[0], cc_sem[1], cc_count)
                    cc_pending.discard(ins.queue_num)
                k = getattr(ins, "_count", None)
                lst = pending_prep.get(ins.queue_num, [])
                pns = lst[:k] if k else lst
                pending_prep[ins.queue_num] = lst[len(pns) :]
                if not pns or pe_sem is None:
                    continue
                tgt = 0
                for pn in pns:
                    nm = prep_info[pn][2]
                    prev = prev_inst.get(nm) if nm else None
                    if prev:
                        tgt = max(tgt, last_reader_n.get(prev, 0))
                if tgt > 0:
                    sid, snm = pe_sem
                    _add_wait(ins, sid, snm, tgt)


def _split_sync_waits(nc, mybir, max_waits=1):
    """This walrus build rejects instructions with more than `max_waits` sync
    waits; hoist excess waits onto injected same-engine InstNoOps."""
    n_split = 0
    for fn in nc.m.functions:
        for bb in fn.blocks:
            out = []
            changed = False
            for ins in bb.instructions:
                si = ins.sync_info
                if si is not None and si.on_wait and len(si.on_wait) > max_waits:
                    waits = list(si.on_wait)
                    excess = waits[:-max_waits]
                    for i in range(0, len(excess), max_waits):
                        nop = mybir.InstNoOp(
                            name=nc.get_next_instruction_name(),
                            sync_info=mybir.SyncInfo(
                                on_wait=excess[i : i + max_waits], on_update=[]
                            ),
                            bass_nofuse=True,
                            engine=ins.engine,
                        )
                        out.append(nop)
                        n_split += 1
                    si.on_wait = waits[-max_waits:]
                    ins.sync_info = si
                    changed = True
                out.append(ins)
            if changed:
                bb.instructions = out
    return n_split


# ----------------------------------------------------------------------------
# Entry point
# ----------------------------------------------------------------------------
def kernel(x, edge_index, W1, b1, W2, b2):
    global LAST_RESULTS
    from concourse.bass_utils import run_bass_kernel_spmd

    x = np.asarray(x)
    W1a = np.asarray(W1)
    b1a = np.asarray(b1)
    W2a = np.asarray(W2)
    b2a = np.asarray(b2)

    key = hash(np.asarray(edge_index)[:, :: E // 997].tobytes())
    if key not in _CACHE:
        plan = _plan(edge_index)
        nc = _build(
            plan["T"],
            plan["CA"],
            plan["CB"],
            plan["groups"],
            plan["tot_chunks"],
            plan["maxc_call"],
        )
        _CACHE[key] = (plan, nc)
    plan, nc = _CACHE[key]

    T = plan["T"]
    SLOTS = plan["SLOTS"]
    NT = NCORES * T

    # xT in table order, tile-major: [NT, 128 infeat, 128 nodes]
    xT = np.zeros((NT, P, P), dtype=np.float16)
    gpos = plan["pos_of"]
    xTflat = np.zeros((P, NCORES * SLOTS), dtype=np.float16)
    xTflat[:, gpos] = x.astype(np.float16).T
    xT[:] = xTflat.reshape(P, NT, P).transpose(1, 0, 2)

    in_common = {
        "xT": xT,
        "W1": W1a.astype(np.float16),
        "W2": W2a.astype(np.float16),
        "b1bc": np.broadcast_to(b1a.astype(np.float32), (P, HID)).copy(),
        "b2bc": np.broadcast_to(b2a.astype(np.float32), (P, OUT_CH)).copy(),
        "ident": np.eye(P, dtype=np.float16),
        "dinv_all": plan["dinv_all"],
        "iotaC": plan["iotaC"],
    }
    in_maps = []
    for c in range(NCORES):
        m = dict(in_common)
        m["colidx"] = plan["colidx_cores"][c]
        m["dinv_own"] = plan["dinv_own_cores"][c]
        m["idx"] = plan["idx_cores"][c]
        in_maps.append(m)

    res = run_bass_kernel_spmd(nc, in_maps, core_ids=list(range(NCORES)))
    LAST_RESULTS = res

    out = np.empty((N, OUT_CH), dtype=np.float32)
    core_of = plan["core_of"]
    slot_of = plan["slot_of"]
    for c in range(NCORES):
        sel = core_of == c
        out[sel] = res.results[c]["out"][slot_of[sel]]
    return out



# revision 19
# speedup vs baseline: 1.9464x; 1.9464x over previous
"""GCN 2-layer encoder on 8 TRN2 NeuronCores (Bass/Tile).

Math (PyG GCNConv, symmetric normalization, self-loops, deg from dst):
    out1 = relu(Dh @ A @ Dh @ (x @ W1) + b1),  Dh = diag(deg^-1/2)
    out  = Dh @ A @ Dh @ (relu1 @ W2) + b2

Factorization (per layer):
    table = Dh @ (feat @ W)               # per-node rows
    agg[d] = table[d] + sum_{e: src->d} table[src]   (self-loop folded out)
    out[d] = dinv[d] * agg[d] + b

Structure on device:
  - Phase 1 (sharded): each core builds only ITS OWN table-1 rows
    (dinv * (x_own @ W1), T tiles) into SBUF + local DRAM, then one
    AllGather replicates the full table1. Own rows stay resident in SBUF
    so the self-loop term is added with one identity matmul per dst tile
    (no self edges in the gather stream).
  - Aggregation: per dst tile, in-edges are packed into 128-lane chunks;
    SWDGE dma_gather fetches message rows from the DRAM table; a per-chunk
    multi-hot sigma (lane -> dst col, built by DVE iota==colidx) scatters
    them on the PE into PSUM. Tables are split in two halves (cores 0-3 /
    4-7) because gather indices are int16.
  - Layer-2 table (relu1*dinv @ W2, 128-padded rows) is built in the
    layer-1 drain, kept in SBUF for the self term, and AllGathered.

Desc-gen pipelining: gather preps (prepare_only) have no real data
dependence (they only read the idx tensor), so the build emits K_PIPE
group-sides of preps ahead of the matching trigger+consume stage, and
post-compile surgery strips the Tile-inserted data waits from the preps.
Correctness moves to the triggers and consumers:
  - triggers are gated on the producing AllGather (Collectives sem) and on
    PE progress (staging-slot WAR, STG_BUFS back);
  - first matmul readers wait on rotating per-queue DMA-completion sems;
  - Tile's vacuous IncSwdgeSem pre-bumps (1.65us of GpSimd each!) are
    deleted outright along with every DMASW lane-sem wait.
This keeps the GpSimd engine desc-genning under phase 1 and under both
collectives instead of idling.
"""

import sys
import types

sys.path.insert(0, "/opt/trn_rl_repo")

import numpy as np

# Register the NTFF profile hook the container's antenv stub lacks, so
# BASS_TRACE=1 profiling works under axon (harmless otherwise).
if "antenv.axon_hooks" not in sys.modules:
    try:
        from trn_agent_boot.trn_boot import _ntff_profile_via_ctypes

        _hook = _ntff_profile_via_ctypes("/opt/axon/libaxon_pjrt.so")
    except Exception:
        _hook = None
    _m = types.ModuleType("antenv.axon_hooks")
    _m.get_axon_ntff_profile_hook = lambda: _hook
    sys.modules["antenv.axon_hooks"] = _m

N = 50000
E = 800000
IN_CH = 128
HID = 128
OUT_CH = 64
NCORES = 8
P = 128
GSZ = 4  # tiles per gather group
CALL_CAP = 8  # max chunks (x128 idxs) per dma_gather call (16KB/engine packet)
SWDGE_QUEUES = 4
BB = 4  # phase-1 DMA batching (tiles per dma_start)
LEAD_Q = 12  # desc-gen lead: max untriggered calls per queue after reorder
CC1_LEAD = 10  # prep units allowed ahead of the first AllGather's issue
PREP_DEPTH = 24  # ring throttle (> LEAD_Q so it never binds in steady state)
STG_BUFS = 3  # staging pool depth (group-sides in flight per tag)
SEM_ROT = 8  # rotating DMA-completion sems per queue

_CACHE = {}
LAST_RESULTS = None


# ----------------------------------------------------------------------------
# Host-side planning
# ----------------------------------------------------------------------------
def _plan(edge_index):
    src = np.asarray(edge_index[0], dtype=np.int64)
    dst = np.asarray(edge_index[1], dtype=np.int64)
    loops = np.arange(N, dtype=np.int64)
    deg = np.bincount(np.concatenate([dst, loops]), minlength=N)
    dinv = (1.0 / np.sqrt(deg.astype(np.float64))).astype(np.float32)

    # node -> core: snake over degree-sorted nodes (balances sum(deg))
    order = np.argsort(-deg, kind="stable")
    snake = np.tile(
        np.concatenate([np.arange(NCORES), np.arange(NCORES - 1, -1, -1)]),
        N // (2 * NCORES) + 1,
    )[:N]
    core_of = np.empty(N, dtype=np.int64)
    core_of[order] = snake

    # per-edge side by src core (self loops are NOT in the stream)
    isA = core_of[src] < (NCORES // 2)
    a_cnt = np.bincount(dst[isA], minlength=N)
    b_cnt = np.bincount(dst[~isA], minlength=N)

    # node -> (tile, col): per core, snake over degree-sorted nodes across
    # provisional tiles (balances per-tile edge sums), tiles then sorted by
    # chunk need desc (aligns profiles across cores) and renumbered.
    tile_of = np.full(N, -1, dtype=np.int64)
    col_of = np.full(N, -1, dtype=np.int64)
    ntiles_max = 0
    prov = []
    for c in range(NCORES):
        nodes = np.where(core_of == c)[0]
        nn = len(nodes)
        ntiles = -(-nn // P)
        ntiles_max = max(ntiles_max, ntiles)
        o2 = np.argsort(-(a_cnt[nodes] + b_cnt[nodes]), kind="stable")
        nds = nodes[o2]
        sn = np.tile(
            np.concatenate([np.arange(ntiles), np.arange(ntiles - 1, -1, -1)]),
            nn // (2 * ntiles) + 1,
        )[:nn]
        prov.append([nds[sn == t] for t in range(ntiles)])

    T = ntiles_max
    ca_t = np.zeros((NCORES, T), dtype=np.int64)
    cb_t = np.zeros((NCORES, T), dtype=np.int64)
    for c in range(NCORES):
        for t, nds in enumerate(prov[c]):
            ca_t[c, t] = -(-int(a_cnt[nds].sum()) // P)
            cb_t[c, t] = -(-int(b_cnt[nds].sum()) // P)
    CA = np.zeros(T, dtype=np.int64)
    CB = np.zeros(T, dtype=np.int64)
    for c in range(NCORES):
        perm = sorted(
            range(len(prov[c])), key=lambda t: -(ca_t[c, t] + cb_t[c, t])
        )
        for p_, t in enumerate(perm):
            nds = prov[c][t]
            tile_of[nds] = p_
            col_of[nds] = np.arange(len(nds))
            CA[p_] = max(CA[p_], ca_t[c, t])
            CB[p_] = max(CB[p_], cb_t[c, t])

    SLOTS = T * P
    HALFROWS = (NCORES // 2) * SLOTS
    assert HALFROWS <= 32768, HALFROWS
    slot_of = tile_of * P + col_of
    pos_of = core_of * SLOTS + slot_of

    ecore = core_of[dst]
    etile = tile_of[dst]
    eside = (~isA).astype(np.int64)
    esrcpos = pos_of[src]
    ecol = col_of[dst]

    G = -(-T // GSZ)
    groups = [list(range(g * GSZ, min((g + 1) * GSZ, T))) for g in range(G)]
    tot_chunks = int(np.sum(CA) + np.sum(CB))
    maxc_call = 0
    for g in groups:
        ca_g = int(sum(CA[p_] for p_ in g))
        cb_g = int(sum(CB[p_] for p_ in g))
        maxc_call = max(maxc_call, ca_g, cb_g)

    ekey = np.lexsort((esrcpos, etile, eside, ecore))
    es_core = ecore[ekey]
    es_side = eside[ekey]
    es_tile = etile[ekey]
    es_srcpos = esrcpos[ekey]
    es_col = ecol[ekey]
    keyv = (es_core * 2 + es_side) * T + es_tile
    uniq, starts = np.unique(keyv, return_index=True)
    ends = np.append(starts[1:], len(keyv))
    bnd = {int(u): (int(s0), int(e0)) for u, s0, e0 in zip(uniq, starts, ends)}

    idx_cores = []
    colidx_cores = []
    dinv_own_cores = []
    for c in range(NCORES):
        flat_idx = []
        flat_col = []
        dvo = np.zeros((P, T), dtype=np.float32)
        nds_c = np.where(core_of == c)[0]
        dvo[col_of[nds_c], tile_of[nds_c]] = dinv[nds_c]

        def emit(side, t, nchunks):
            k = (c * 2 + side) * T + t
            s0, e0 = bnd.get(k, (0, 0))
            sp = es_srcpos[s0:e0]
            cl = es_col[s0:e0]
            if side == 1:
                sp = sp - HALFROWS
            n_ = e0 - s0
            want = nchunks * P
            # pad lanes: idx 0 (any valid row) with col -1 -> sigma row is
            # all-zero, so the gathered garbage is multiplied by 0.
            ii = np.zeros(want, np.int64)
            cc = np.full(want, -1.0, np.float64)
            ii[:n_] = sp
            cc[:n_] = cl
            flat_idx.append(ii)
            flat_col.append(cc)

        for g in groups:
            for p_ in g:
                emit(0, p_, int(CA[p_]))
            for p_ in g:
                emit(1, p_, int(CB[p_]))
        fi = np.concatenate(flat_idx)
        fc = np.concatenate(flat_col)
        assert fi.size == tot_chunks * P
        assert fi.min() >= 0 and fi.max() < HALFROWS
        wrapped = fi.astype(np.int16).reshape(-1, 16).T.copy()
        idx_cores.append(np.tile(wrapped, (8, 1)))
        colidx_cores.append(fc.reshape(tot_chunks, P).T.astype(np.float16).copy())
        dinv_own_cores.append(dvo)

    iotaC = np.tile(
        np.arange(P, dtype=np.float16)[None, :], (P, maxc_call)
    ).reshape(P, maxc_call * P)

    return dict(
        T=T,
        SLOTS=SLOTS,
        CA=CA,
        CB=CB,
        groups=groups,
        tot_chunks=tot_chunks,
        maxc_call=maxc_call,
        core_of=core_of,
        slot_of=slot_of,
        pos_of=pos_of,
        dinv=dinv,
        idx_cores=idx_cores,
        colidx_cores=colidx_cores,
        dinv_own_cores=dinv_own_cores,
        iotaC=iotaC,
    )


# ----------------------------------------------------------------------------
# Device kernel
# ----------------------------------------------------------------------------
def _build(
    T,
    CA,
    CB,
    groups,
    tot_chunks,
    maxc_call,
    detect_races=True,
):
    import concourse.bass as bass
    import concourse.mybir as mybir
    import concourse.tile as tile
    from concourse import bacc

    f16 = mybir.dt.float16
    f32 = mybir.dt.float32
    i16 = mybir.dt.int16
    SLOTS = T * P
    ROWS = NCORES * SLOTS
    HALFROWS = ROWS // 2

    nc = bacc.Bacc(
        "TRN2",
        target_bir_lowering=False,
        num_devices=NCORES,
        num_swdge_queues=SWDGE_QUEUES,
        detect_race_conditions=detect_races,
    )
    qn = [0]

    def _next_q():
        qn[0] = (qn[0] + 1) % SWDGE_QUEUES
        return qn[0]

    dma_sems = [
        [nc.alloc_semaphore(f"swdge_dma_q{i}r{r}") for r in range(SEM_ROT)]
        for i in range(SWDGE_QUEUES)
    ]
    q_calls = [0] * SWDGE_QUEUES

    def _prep(out_ap, in_ap, idx_ap, n_idx, q):
        jq = q_calls[q]
        q_calls[q] += 1
        nc.gpsimd.dma_gather(
            out_ap,
            in_ap,
            idx_ap,
            n_idx,
            n_idx,
            P,
            prepare_only=True,
            sem=dma_sems[q][jq % SEM_ROT],
            queue_num=q,
        )

    xT_in = nc.dram_tensor("xT", [T, P, P], f16, kind="ExternalInput")
    w1_in = nc.dram_tensor("W1", [IN_CH, HID], f16, kind="ExternalInput")
    w2_in = nc.dram_tensor("W2", [HID, OUT_CH], f16, kind="ExternalInput")
    b1_in = nc.dram_tensor("b1bc", [P, HID], f32, kind="ExternalInput")
    b2_in = nc.dram_tensor("b2bc", [P, OUT_CH], f32, kind="ExternalInput")
    id_in = nc.dram_tensor("ident", [P, P], f16, kind="ExternalInput")
    col_in = nc.dram_tensor("colidx", [P, tot_chunks], f16, kind="ExternalInput")
    iota_in = nc.dram_tensor("iotaC", [P, maxc_call * P], f16, kind="ExternalInput")
    do_in = nc.dram_tensor("dinv_own", [P, T], f32, kind="ExternalInput")
    idx_in = nc.dram_tensor("idx", [P, tot_chunks * 8], i16, kind="ExternalInput")
    out_ext = nc.dram_tensor("out", [SLOTS, OUT_CH], f32, kind="ExternalOutput")

    with tile.TileContext(nc) as tc:
        with (
            tc.tile_pool(name="const", bufs=1) as cpool,
            tc.tile_pool(name="xt", bufs=3) as xtpool,
            tc.tile_pool(name="sig", bufs=3) as sigpool,
            tc.tile_pool(name="stg", bufs=STG_BUFS) as stgpool,
            tc.tile_pool(name="drain", bufs=3) as dpool,
            tc.tile_pool(name="psb", bufs=2, space="PSUM") as ps_build,
            tc.tile_pool(name="psa", bufs=3, space="PSUM") as ps_agg,
            tc.tile_pool(name="pst", bufs=2, space="PSUM") as ps_tr,
            tc.tile_pool(name="psm", bufs=1, space="PSUM") as ps_mm2,
            tc.tile_pool(name="dram", bufs=1, space="DRAM") as dram,
        ):
            # ---- constants into SBUF (idx first: it gates desc-gen) ----
            idx_sb = cpool.tile([P, tot_chunks * 8], i16)
            nc.sync.dma_start(out=idx_sb[:], in_=idx_in[:])
            w1_sb = cpool.tile([IN_CH, HID], f16)
            nc.sync.dma_start(out=w1_sb[:], in_=w1_in[:])
            w2_sb = cpool.tile([HID, OUT_CH], f16)
            nc.sync.dma_start(out=w2_sb[:], in_=w2_in[:])
            b1_sb = cpool.tile([P, HID], f32)
            nc.sync.dma_start(out=b1_sb[:], in_=b1_in[:])
            b2_sb = cpool.tile([P, OUT_CH], f32)
            nc.sync.dma_start(out=b2_sb[:], in_=b2_in[:])
            id_sb = cpool.tile([P, P], f16)
            nc.sync.dma_start(out=id_sb[:], in_=id_in[:])
            col_sb = cpool.tile([P, tot_chunks], f16)
            nc.sync.dma_start(out=col_sb[:], in_=col_in[:])
            iota_sb = cpool.tile([P, maxc_call * P], f16)
            nc.sync.dma_start(out=iota_sb[:], in_=iota_in[:])
            do_sb = cpool.tile([P, T], f32)
            nc.sync.dma_start(out=do_sb[:], in_=do_in[:])

            # own table rows stay in SBUF for the self-loop matmul
            own1_sb = cpool.tile([P, T * HID], f16)
            own2_sb = cpool.tile([P, T * OUT_CH], f16)

            own1 = dram.tile([SLOTS, HID], f16)
            table1 = dram.tile([ROWS, HID], f16, addr_space="Shared")
            shard2 = dram.tile([SLOTS, P], f16)
            table2 = dram.tile([ROWS, P], f16, addr_space="Shared")

            # ---- phase 1 (sharded): own1 = dinv_own * (x_own @ W1) ----
            for j0 in range(0, T, BB):
                nb = min(BB, T - j0)
                xt_t = xtpool.tile([P, nb * P], f16, tag="xt")
                nc.sync.dma_start(
                    out=xt_t[:].rearrange("p (t c) -> p t c", t=nb),
                    in_=xT_in[j0 : j0 + nb].rearrange("t p c -> p t c"),
                )
                for k in range(nb):
                    j = j0 + k
                    bps = ps_build.tile([P, HID], f32, tag="build")
                    nc.tensor.matmul(
                        bps[:],
                        lhsT=xt_t[:, k * P : (k + 1) * P],
                        rhs=w1_sb[:],
                        start=True,
                        stop=True,
                    )
                    nc.scalar.activation(
                        own1_sb[:, j * HID : (j + 1) * HID],
                        bps[:],
                        mybir.ActivationFunctionType.Copy,
                        scale=do_sb[:, j : j + 1],
                    )
                nc.sync.dma_start(
                    out=own1[j0 * P : (j0 + nb) * P, :].rearrange(
                        "(t p) f -> p t f", t=nb
                    ),
                    in_=own1_sb[:, j0 * HID : (j0 + nb) * HID].rearrange(
                        "p (t f) -> p t f", t=nb
                    ),
                )

            # ---- drains ----
            def drain(layer, p_, aps):
                dv = do_sb[:, p_ : p_ + 1]
                if layer == 0:
                    r1 = dpool.tile([P, HID], f32, tag="r1")
                    nc.scalar.activation(
                        r1[:], aps[:], mybir.ActivationFunctionType.Copy, scale=dv
                    )
                    nc.vector.tensor_add(r1[:], r1[:], b1_sb[:])
                    r3 = dpool.tile([P, HID], f16, tag="r3")
                    nc.scalar.activation(
                        r3[:], r1[:], mybir.ActivationFunctionType.Relu, scale=dv
                    )
                    psT = ps_tr.tile([P, P], f16, tag="tr")
                    nc.tensor.transpose(psT[:], r3[:], id_sb[:])
                    rT = dpool.tile([P, P], f16, tag="rT")
                    nc.scalar.activation(
                        rT[:], psT[:], mybir.ActivationFunctionType.Copy
                    )
                    ps2 = ps_mm2.tile([P, OUT_CH], f32, tag="mm2")
                    nc.tensor.matmul(
                        ps2[:], lhsT=rT[:], rhs=w2_sb[:], start=True, stop=True
                    )
                    nc.scalar.activation(
                        own2_sb[:, p_ * OUT_CH : (p_ + 1) * OUT_CH],
                        ps2[:],
                        mybir.ActivationFunctionType.Copy,
                    )
                    t2 = dpool.tile([P, P], f16, tag="t2")
                    nc.scalar.activation(
                        t2[:, 0:OUT_CH], ps2[:], mybir.ActivationFunctionType.Copy
                    )
                    nc.vector.memset(t2[:, OUT_CH:P], 0.0)
                    nc.sync.dma_start(
                        out=shard2[p_ * P : (p_ + 1) * P, :], in_=t2[:]
                    )
                else:
                    o1 = dpool.tile([P, OUT_CH], f32, tag="o1")
                    nc.scalar.activation(
                        o1[:], aps[:], mybir.ActivationFunctionType.Copy, scale=dv
                    )
                    nc.vector.tensor_add(o1[:], o1[:], b2_sb[:])
                    nc.sync.dma_start(
                        out=out_ext[p_ * P : (p_ + 1) * P, :], in_=o1[:]
                    )

            # ---- aggregation ----
            def aggregate(layer):
                tab = table1 if layer == 0 else table2
                nfeat = HID if layer == 0 else OUT_CH
                own_sb = own1_sb if layer == 0 else own2_sb
                ownw = HID if layer == 0 else OUT_CH

                sides = []
                coff = 0
                for gi, g in enumerate(groups):
                    for side in (0, 1):
                        cnt = CA if side == 0 else CB
                        c_g = int(sum(int(cnt[p_]) for p_ in g))
                        sides.append((gi, side, coff, c_g))
                        coff += c_g
                staged = {}

                def stage(i):
                    gi, side, soff, c_g = sides[i]
                    if c_g == 0:
                        staged[i] = (None, None, [])
                        return
                    if side == 0:
                        stA = stgpool.tile([P, maxc_call, P], f16, tag="stgA")
                        st = stA
                    else:
                        stB = stgpool.tile([P, maxc_call, P], f16, tag="stgB")
                        st = stB
                    percall = []
                    for s_ in range(0, c_g, CALL_CAP):
                        n_ = min(CALL_CAP, c_g - s_)
                        q = _next_q()
                        _prep(
                            st[:, s_ : s_ + n_, :],
                            tab[0:HALFROWS, :]
                            if side == 0
                            else tab[HALFROWS:ROWS, :],
                            idx_sb[:, (soff + s_) * 8 : (soff + s_ + n_) * 8],
                            n_ * P,
                            q,
                        )
                        percall.append(q)
                    cnts = {}
                    for q in percall:
                        cnts[q] = cnts.get(q, 0) + 1
                    if side == 0:
                        sgA = sigpool.tile([P, maxc_call * P], f16, tag="sgA")
                        sg = sgA
                    else:
                        sgB = sigpool.tile([P, maxc_call * P], f16, tag="sgB")
                        sg = sgB
                    nc.vector.tensor_tensor(
                        sg[:, : c_g * P].rearrange("p (k c) -> p k c", k=c_g),
                        iota_sb[:, : c_g * P].rearrange("p (k c) -> p k c", k=c_g),
                        col_sb[:, soff : soff + c_g]
                        .unsqueeze(-1)
                        .broadcast_to([P, c_g, P]),
                        mybir.AluOpType.is_equal,
                    )
                    staged[i] = (st, sg, sorted(cnts.items()))

                def fire(i):
                    for q, k in staged[i][2]:
                        nc.gpsimd.trigger_dma(count=k, queue_num=q)

                def consume(gi):
                    stA, sgA_, _ = staged[2 * gi]
                    stB, sgB_, _ = staged[2 * gi + 1]
                    a_off = 0
                    b_off = 0
                    for p_ in groups[gi]:
                        aps = ps_agg.tile([P, nfeat], f32, tag="agg")
                        k = 0
                        for ci in range(int(CA[p_])):
                            cc = a_off + ci
                            nc.tensor.matmul(
                                aps[:],
                                lhsT=sgA_[:, cc * P : (cc + 1) * P],
                                rhs=stA[:, cc, 0:nfeat],
                                start=(k == 0),
                                stop=False,
                            )
                            k += 1
                        for ci in range(int(CB[p_])):
                            cc = b_off + ci
                            nc.tensor.matmul(
                                aps[:],
                                lhsT=sgB_[:, cc * P : (cc + 1) * P],
                                rhs=stB[:, cc, 0:nfeat],
                                start=(k == 0),
                                stop=False,
                            )
                            k += 1
                        # self-loop row: aps += I @ own_row
                        nc.tensor.matmul(
                            aps[:],
                            lhsT=id_sb[:],
                            rhs=own_sb[:, p_ * ownw : (p_ + 1) * ownw],
                            start=(k == 0),
                            stop=True,
                        )
                        a_off += int(CA[p_])
                        b_off += int(CB[p_])
                        drain(layer, p_, aps)

                # Emission is NOT pipelined: each trigger immediately follows
                # its side's preps so Tile's no_sync prep->trigger links bind
                # exactly (a trigger with an empty pending list free-floats
                # and gets hoisted by the scheduler). The desc-gen pipelining
                # is done post-compile by _reorder_pool.
                for i in range(len(sides)):
                    stage(i)
                    fire(i)
                    if i % 2 == 1:
                        consume(i // 2)

            def cc1():
                nc.gpsimd.collective_compute(
                    "AllGather",
                    mybir.AluOpType.bypass,
                    replica_groups=[list(range(NCORES))],
                    ins=[own1.opt()],
                    outs=[table1.opt()],
                )

            def cc2():
                nc.gpsimd.collective_compute(
                    "AllGather",
                    mybir.AluOpType.bypass,
                    replica_groups=[list(range(NCORES))],
                    ins=[shard2.opt()],
                    outs=[table2.opt()],
                )

            cc1()
            aggregate(0)
            cc2()
            aggregate(1)

    nc.compile()  # bacc passes: library loads, register allocation, DCE
    _strip_tile_sync(nc, mybir)
    _reorder_pool(nc, mybir)
    _fix_swdge_sems(nc, mybir)
    _split_sync_waits(nc, mybir, max_waits=1)
    return nc


def _reorder_pool(nc, mybir):
    """Software-pipeline desc-gen by permuting the Pool subsequence.

    The scheduled stream has each gather prep just before its trigger. Hoist
    prep units ([RegisterMove, DMAGatherAnt] pairs) earlier among the Pool
    instructions — preserving prep relative order (ring FIFO order and Pool
    tick counts depend on it) and the relative order of everything else —
    so desc-gen runs ahead of the trigger/consume stages, bounded by LEAD_Q
    untriggered calls per queue. At most CC1_LEAD units go ahead of the
    first AllGather so its issue (which blocks the in-order Pool stream on
    the phase-1 writes) isn't pushed behind ~100us of desc-gen.
    """
    for fn in nc.m.functions:
        for bb in fn.blocks:
            insts = bb.instructions
            pool_pos = [
                i
                for i, ins in enumerate(insts)
                if getattr(ins, "engine", None) == mybir.EngineType.Pool
            ]
            if not pool_pos:
                continue
            pool = [insts[i] for i in pool_pos]
            # Pair each prep with the RegisterMove writing its num_idxs reg
            # (regrefs are single-assignment, one write per prep; the pair
            # may be separated by EventSemaphores in the scheduled stream).
            import re as _re

            def _regref(ap_str):
                m = _re.search(r"regref='([^']+)'", ap_str)
                return m.group(1) if m else None

            reg_writer = {}
            for ins in pool:
                if type(ins).__name__ == "InstRegisterMove" and ins.outs:
                    rr = _regref(str(ins.outs[0]))
                    if rr is not None:
                        assert rr not in reg_writer, rr
                        reg_writer[rr] = ins
            consumed = set()
            units = []  # (queue, [regmove, prep])
            others = []
            for ins in pool:
                tn = type(ins).__name__
                if tn == "InstDMAGatherAnt" and getattr(ins, "gen_mode", 0) == 1:
                    rr = _regref(str(ins.ins[-1]))
                    rm = reg_writer.get(rr)
                    assert rm is not None, rr
                    consumed.add(id(rm))
                    units.append((ins.queue_num, [rm, ins]))
            for ins in pool:
                tn = type(ins).__name__
                if tn == "InstDMAGatherAnt" and getattr(ins, "gen_mode", 0) == 1:
                    continue
                if id(ins) in consumed:
                    continue
                others.append(ins)
            if not units:
                continue
            # The SWDGE ucode library must be loaded before ANY gpsimd
            # library op (gather desc-gen, collective): hoist the reload to
            # the front of this block's Pool stream.
            relo = [
                o
                for o in others
                if type(o).__name__ == "InstPseudoReloadLibraryIndex"
            ]
            if relo:
                others.remove(relo[0])
                others.insert(0, relo[0])
            new_pool = []
            inflight = {}
            emitted = {}
            fired = {}
            ucur = 0
            ncc = 0
            # no desc-gen before the SWDGE ucode library reload
            lib_done = not any(
                type(o).__name__ == "InstPseudoReloadLibraryIndex" for o in others
            )

            def can_emit():
                if ucur >= len(units):
                    return False
                q = units[ucur][0]
                return inflight.get(q, 0) < LEAD_Q

            def emit_unit():
                nonlocal ucur
                q, ins_list = units[ucur]
                new_pool.extend(ins_list)
                inflight[q] = inflight.get(q, 0) + 1
                emitted[q] = emitted.get(q, 0) + 1
                ucur += 1

            for ins in others:
                tn = type(ins).__name__
                if not lib_done:
                    new_pool.append(ins)
                    if tn == "InstPseudoReloadLibraryIndex":
                        lib_done = True
                    continue
                if tn == "InstTriggerDma":
                    q = ins.queue_num
                    k = getattr(ins, "_count", None) or 0
                    # force this trigger's calls into the stream first
                    while emitted.get(q, 0) < fired.get(q, 0) + k:
                        assert ucur < len(units)
                        emit_unit()
                    fired[q] = fired.get(q, 0) + k
                    inflight[q] = emitted.get(q, 0) - fired.get(q, 0)
                # greedy hoist, capped before the first collective
                while can_emit() and not (ncc == 0 and ucur >= CC1_LEAD):
                    emit_unit()
                new_pool.append(ins)
                if tn == "InstCollectiveCompute":
                    ncc += 1
            while ucur < len(units):
                emit_unit()
            assert len(new_pool) == len(pool)
            for pos, ins in zip(pool_pos, new_pool):
                insts[pos] = ins
            bb.instructions = insts


def _add_wait(ins, mybir, sid, snm, val):
    si = ins.sync_info
    if si is None:
        si = mybir.SyncInfo(on_wait=[], on_update=[])
    si.on_wait = list(si.on_wait or []) + [
        mybir.SyncWait(
            sync_type="semaphore",
            id=sid,
            ant_name=snm,
            wait_mode="sem-ge-imm",
            wait_value=val,
            wait_reg=None,
        )
    ]
    ins.sync_info = si


def _strip_tile_sync(nc, mybir):
    """Delete IncSwdgeSem pre-bumps + every DMASW lane-sem wait; clear all
    waits from gather preps (re-adding an idx-load gate on the first one);
    clear data waits from Pool-stream gate instructions (EventSemaphore /
    NoOp) up to the last trigger so desc-gen free-runs."""
    streams = []
    for fn in nc.m.functions:
        for bb in fn.blocks:
            streams.append(bb)
    # DMAHW completion gates: cumulative per-lane counts at the idx load and
    # at each write into the collectives' source tensors (Tile's own gates
    # for these were hoisted onto Pool EventSemaphores, which get stripped).
    idx_gate = None
    cum = {}
    sem_ids = {}
    thr = {"own1": {}, "shard2": {}}
    for bb in streams:
        for ins in bb.instructions:
            if type(ins).__name__ != "InstDMACopy":
                continue
            si = ins.sync_info
            for u in (si.on_update or []) if si else []:
                nm = u.ant_name or ""
                if nm.startswith("DMAHW"):
                    cum[nm] = cum.get(nm, 0) + (u.update_value or 16)
                    sem_ids[nm] = u.id
                    outs = str(ins.outs[0])
                    if idx_gate is None and "idx_sb" in outs:
                        idx_gate = (u.id, nm, cum[nm])
                    for t in ("own1", "shard2"):
                        if f"{t}_" in outs or f"'{t}'" in outs:
                            thr[t][nm] = cum[nm]

    # last trigger position per bb for the Pool-gate strip range
    last_trig = {}
    for bi, bb in enumerate(streams):
        for pos, ins in enumerate(bb.instructions):
            if type(ins).__name__ == "InstTriggerDma":
                last_trig[bi] = pos

    first_prep = None
    n_inc = 0
    for bi, bb in enumerate(streams):
        out = []
        for pos, ins in enumerate(bb.instructions):
            tn = type(ins).__name__
            if tn == "InstIncSwdgeSem":
                n_inc += 1
                continue
            si = ins.sync_info
            if si and si.on_wait:
                w2 = [
                    w
                    for w in si.on_wait
                    if not (w.ant_name or "").startswith("DMASW")
                ]
                if tn == "InstDMAGatherAnt" and getattr(ins, "gen_mode", 0) == 1:
                    w2 = []
                elif (
                    ins.engine == mybir.EngineType.Pool
                    and tn in ("InstEventSemaphore", "InstNoOp")
                    and pos <= last_trig.get(bi, -1)
                ):
                    w2 = [
                        w
                        for w in w2
                        if (w.ant_name or "").startswith(
                            ("barrier_", "swdge_dma_")
                        )
                    ]
                si.on_wait = w2
            if (
                tn == "InstDMAGatherAnt"
                and getattr(ins, "gen_mode", 0) == 1
                and first_prep is None
            ):
                first_prep = ins
            out.append(ins)
        bb.instructions = out
    assert n_inc > 0
    assert first_prep is not None
    if idx_gate is not None:
        _add_wait(first_prep, mybir, idx_gate[0], idx_gate[1], idx_gate[2])

    # re-gate the collectives on their source tensors' write completion
    assert thr["own1"] and thr["shard2"], thr
    n_cc = 0
    for bb in streams:
        for ins in bb.instructions:
            if type(ins).__name__ != "InstCollectiveCompute":
                continue
            t = "own1" if n_cc == 0 else "shard2"
            n_cc += 1
            have = {
                (w.ant_name, w.wait_value) for w in (ins.sync_info.on_wait or [])
            }
            for nm, v in sorted(thr[t].items()):
                if (nm, v) not in have:
                    _add_wait(ins, mybir, sem_ids[nm], nm, v)
    assert n_cc == 2, n_cc


def _fix_swdge_sems(nc, mybir):
    """Sem rewiring for the stripped, reordered prep/trigger SWDGE path.

    1. Data RAW: the first matmul reading each staging-tile instance waits
       on every covering gather call: sem_q(rot) >= 16*(call# in rot + 1).
    2. WAR + collective gating on triggers: a trigger firing a DMA that
       overwrites a staging slot waits on PE progress (last reader of the
       instance STG_BUFS allocations back); the first trigger per queue
       after each AllGather waits on its completion.
    3. Ring throttle: prep #j on queue q waits sem_q so at most PREP_DEPTH
       calls per queue are in flight (untriggered ring-entry cap).
    """
    import re

    streams = []
    for fn in nc.m.functions:
        for bb in fn.blocks:
            streams.append(bb)

    queue_sems = {}
    pe_sem = None
    cc_sem = None
    for bb in streams:
        for ins in bb.instructions:
            si = ins.sync_info
            if not si:
                continue
            for u in si.on_update or []:
                nm = u.ant_name or ""
                if nm.startswith("swdge_dma_q"):
                    qs, rs = nm[11:].split("r")
                    queue_sems[(int(qs), int(rs))] = (u.id, nm)
            for w in si.on_wait or []:
                nm = w.ant_name or ""
                if nm.startswith("PE_") and pe_sem is None:
                    pe_sem = (w.id, nm)
                if nm.startswith("Collectives") and cc_sem is None:
                    cc_sem = (w.id, nm)
            if type(ins).__name__ == "InstCollectiveCompute" and cc_sem is None:
                for u in si.on_update or []:
                    if (u.ant_name or "").startswith("Collectives"):
                        cc_sem = (u.id, u.ant_name)
    assert pe_sem is not None and cc_sem is not None, (pe_sem, cc_sem)

    pat = re.compile(r"\b(st[AB])_(\d+)\b")

    def _stg_name(ap):
        m = pat.search(str(ap))
        return m.group(0) if m else None

    # pass 1 prep: per-queue call indices; staging instances: creation order
    # (per tag), covering calls, first/last matmul readers
    inst_order = {"stA": [], "stB": []}
    seen = set()
    first_reader = {}
    last_reader_n = {}
    inst_calls = {}
    prep_info = {}
    q_count = {}
    pe_n = 0
    for bb in streams:
        for ins in bb.instructions:
            tn = type(ins).__name__
            if tn == "InstMatmult":
                pe_n += 1
                for ap in ins.ins or []:
                    nm = _stg_name(ap)
                    if nm:
                        if nm not in first_reader:
                            first_reader[nm] = ins
                        last_reader_n[nm] = pe_n
            elif tn == "InstDMAGatherAnt" and getattr(ins, "gen_mode", 0) == 1:
                q = ins.queue_num
                jq = q_count.get(q, 0)
                q_count[q] = jq + 1
                nm = _stg_name(ins.outs[0])
                prep_info[ins.name] = (q, jq, nm)
                if nm:
                    inst_calls.setdefault(nm, []).append((q, jq))
                    if nm not in seen:
                        seen.add(nm)
                        inst_order[nm[:3]].append(nm)
    assert first_reader, "staging-name regex matched nothing"
    prev_inst = {}
    for tag, lst in inst_order.items():
        for i, nm in enumerate(lst):
            if i >= STG_BUFS:
                prev_inst[nm] = lst[i - STG_BUFS]

    # 1. data RAW waits on first readers (per (queue, rotation) max target)
    for nm, rd in first_reader.items():
        per_qr = {}
        for q, jq in inst_calls.get(nm, []):
            k = (q, jq % SEM_ROT)
            per_qr[k] = max(per_qr.get(k, -1), jq // SEM_ROT)
        for k, t in sorted(per_qr.items()):
            if k in queue_sems:
                sid, snm = queue_sems[k]
                _add_wait(rd, mybir, sid, snm, 16 * (t + 1))

    # 2 + 3. WAR + collective gating on triggers, throttle on preps
    pending_prep = {}
    cc_count = 0
    cc_pending = set()
    for bb in streams:
        for ins in bb.instructions:
            tn = type(ins).__name__
            if tn == "InstCollectiveCompute":
                cc_count += 1
                cc_pending = set(range(SWDGE_QUEUES))
            elif tn == "InstDMAGatherAnt" and getattr(ins, "gen_mode", 0) == 1:
                q, jq, nm = prep_info[ins.name]
                pending_prep.setdefault(q, []).append(ins.name)
                if jq >= PREP_DEPTH:
                    jt = jq - PREP_DEPTH
                    k = (q, jt % SEM_ROT)
                    if k in queue_sems:
                        sid, snm = queue_sems[k]
                        _add_wait(ins, mybir, sid, snm, 16 * (jt // SEM_ROT + 1))
            elif tn == "InstTriggerDma":
                if ins.queue_num in cc_pending and cc_sem is not None:
                    _add_wait(ins, mybir, cc_sem[0], cc_sem[1], cc_count)
                    cc_pending.discard(ins.queue_num)
                k = getattr(ins, "_count", None)
                lst = pending_prep.get(ins.queue_num, [])
                pns = lst[:k] if k else lst
                pending_prep[ins.queue_num] = lst[len(pns) :]
                if not pns or pe_sem is None:
                    continue
                tgt = 0
                for pn in pns:
                    nm = prep_info[pn][2]
                    prev = prev_inst.get(nm) if nm else None
                    if prev:
                        tgt = max(tgt, last_reader_n.get(prev, 0))
                if tgt > 0:
                    sid, snm = pe_sem
                    _add_wait(ins, mybir, sid, snm, tgt)


def _split_sync_waits(nc, mybir, max_waits=1):
    """This walrus build rejects instructions with more than `max_waits` sync
    waits; hoist excess waits onto injected same-engine InstNoOps."""
    n_split = 0
    for fn in nc.m.functions:
        for bb in fn.blocks:
            out = []
            changed = False
            for ins in bb.instructions:
                si = ins.sync_info
                if si is not None and si.on_wait and len(si.on_wait) > max_waits:
                    waits = list(si.on_wait)
                    excess = waits[:-max_waits]
                    for i in range(0, len(excess), max_waits):
                        nop = mybir.InstNoOp(
                            name=nc.get_next_instruction_name(),
                            sync_info=mybir.SyncInfo(
                                on_wait=excess[i : i + max_waits], on_update=[]
                            ),
                            bass_nofuse=True,
                            engine=ins.engine,
                        )
                        out.append(nop)
                        n_split += 1
                    si.on_wait = waits[-max_waits:]
                    ins.sync_info = si
                    changed = True
                out.append(ins)
            if changed:
                bb.instructions = out
    return n_split


# ----------------------------------------------------------------------------
# Entry point
# ----------------------------------------------------------------------------
def kernel(x, edge_index, W1, b1, W2, b2):
    global LAST_RESULTS
    from concourse.bass_utils import run_bass_kernel_spmd

    x = np.asarray(x)
    W1a = np.asarray(W1)
    b1a = np.asarray(b1)
    W2a = np.asarray(W2)
    b2a = np.asarray(b2)

    key = hash(np.asarray(edge_index)[:, :: E // 997].tobytes())
    if key not in _CACHE:
        plan = _plan(edge_index)
        nc = _build(
            plan["T"],
            plan["CA"],
            plan["CB"],
            plan["groups"],
            plan["tot_chunks"],
            plan["maxc_call"],
        )
        _CACHE[key] = (plan, nc)
    plan, nc = _CACHE[key]

    T = plan["T"]
    SLOTS = plan["SLOTS"]

    core_of = plan["core_of"]
    slot_of = plan["slot_of"]

    in_common = {
        "W1": W1a.astype(np.float16),
        "W2": W2a.astype(np.float16),
        "b1bc": np.broadcast_to(b1a.astype(np.float32), (P, HID)).copy(),
        "b2bc": np.broadcast_to(b2a.astype(np.float32), (P, OUT_CH)).copy(),
        "ident": np.eye(P, dtype=np.float16),
        "iotaC": plan["iotaC"],
    }
    in_maps = []
    xf16 = x.astype(np.float16)
    for c in range(NCORES):
        sel = core_of == c
        nodes = np.where(sel)[0]
        xTflat = np.zeros((P, SLOTS), dtype=np.float16)
        xTflat[:, slot_of[nodes]] = xf16[nodes].T
        xTc = xTflat.reshape(P, T, P).transpose(1, 0, 2).copy()
        m = dict(in_common)
        m["xT"] = xTc
        m["colidx"] = plan["colidx_cores"][c]
        m["dinv_own"] = plan["dinv_own_cores"][c]
        m["idx"] = plan["idx_cores"][c]
        in_maps.append(m)

    res = run_bass_kernel_spmd(nc, in_maps, core_ids=list(range(NCORES)))
    LAST_RESULTS = res

    out = np.empty((N, OUT_CH), dtype=np.float32)
    for c in range(NCORES):
        sel = core_of == c
        out[sel] = res.results[c]["out"][slot_of[sel]]
    return out


# revision 22
# speedup vs baseline: 2.5186x; 1.2940x over previous
"""GCN 2-layer encoder on 8 TRN2 NeuronCores (Bass/Tile).

Math (PyG GCNConv, symmetric normalization, self-loops, deg from dst):
    out1 = relu(Dh @ A @ Dh @ (x @ W1) + b1),  Dh = diag(deg^-1/2)
    out  = Dh @ A @ Dh @ (relu1 @ W2) + b2

Factorization (per layer):
    table = Dh @ (feat @ W)               # per-node rows
    agg[d] = table[d] + sum_{e: src->d} table[src]   (self-loop folded out)
    out[d] = dinv[d] * agg[d] + b

Structure on device:
  - Phase 1 (sharded): each core builds only ITS OWN table-1 rows
    (dinv * (x_own @ W1), T tiles) into SBUF + local DRAM, then one
    AllGather replicates the full table1. Own rows stay resident in SBUF
    so the self-loop term is added with one identity matmul per dst tile
    (no self edges in the gather stream).
  - Aggregation: per dst tile, in-edges are packed into 128-lane chunks;
    SWDGE dma_gather fetches message rows from the DRAM table; a per-chunk
    multi-hot sigma (lane -> dst col, built by DVE iota==colidx) scatters
    them on the PE into PSUM. Tables are split in two halves (cores 0-3 /
    4-7) because gather indices are int16.
  - Layer-2 table (relu1*dinv @ W2, 128-padded rows) is built in the
    layer-1 drain, kept in SBUF for the self term, and AllGathered.

Desc-gen pipelining: gather preps (prepare_only) have no real data
dependence (they only read the idx tensor), so the build emits K_PIPE
group-sides of preps ahead of the matching trigger+consume stage, and
post-compile surgery strips the Tile-inserted data waits from the preps.
Correctness moves to the triggers and consumers:
  - triggers are gated on the producing AllGather (Collectives sem) and on
    PE progress (staging-slot WAR, STG_BUFS back);
  - first matmul readers wait on rotating per-queue DMA-completion sems;
  - Tile's vacuous IncSwdgeSem pre-bumps (1.65us of GpSimd each!) are
    deleted outright along with every DMASW lane-sem wait.
This keeps the GpSimd engine desc-genning under phase 1 and under both
collectives instead of idling.
"""

import sys
import types

sys.path.insert(0, "/opt/trn_rl_repo")

import numpy as np

# Register the NTFF profile hook the container's antenv stub lacks, so
# BASS_TRACE=1 profiling works under axon (harmless otherwise).
if "antenv.axon_hooks" not in sys.modules:
    try:
        from trn_agent_boot.trn_boot import _ntff_profile_via_ctypes

        _hook = _ntff_profile_via_ctypes("/opt/axon/libaxon_pjrt.so")
    except Exception:
        _hook = None
    _m = types.ModuleType("antenv.axon_hooks")
    _m.get_axon_ntff_profile_hook = lambda: _hook
    sys.modules["antenv.axon_hooks"] = _m

N = 50000
E = 800000
IN_CH = 128
HID = 128
OUT_CH = 64
NCORES = 8
P = 128
GSZ = 4  # tiles per gather group
CALL_CAP = 8  # max chunks (x128 idxs) per dma_gather call (16KB/engine packet)
SWDGE_QUEUES = 4
BB = 4  # phase-1 DMA batching (tiles per dma_start)
LEAD_Q = 12  # desc-gen lead: max untriggered calls per queue after reorder
CC1_LEAD = 10  # prep units allowed ahead of the first AllGather's issue
PREP_DEPTH = 24  # ring throttle (> LEAD_Q so it never binds in steady state)
STG_BUFS = 4  # staging pool depth (group-sides in flight per tag)
SEM_ROT = 8  # rotating DMA-completion sems per queue

_CACHE = {}
LAST_RESULTS = None


# ----------------------------------------------------------------------------
# Host-side planning
# ----------------------------------------------------------------------------
def _plan(edge_index):
    src = np.asarray(edge_index[0], dtype=np.int64)
    dst = np.asarray(edge_index[1], dtype=np.int64)
    loops = np.arange(N, dtype=np.int64)
    deg = np.bincount(np.concatenate([dst, loops]), minlength=N)
    dinv = (1.0 / np.sqrt(deg.astype(np.float64))).astype(np.float32)

    # node -> core: snake over degree-sorted nodes (balances sum(deg))
    order = np.argsort(-deg, kind="stable")
    snake = np.tile(
        np.concatenate([np.arange(NCORES), np.arange(NCORES - 1, -1, -1)]),
        N // (2 * NCORES) + 1,
    )[:N]
    core_of = np.empty(N, dtype=np.int64)
    core_of[order] = snake

    # per-edge side by src core (self loops are NOT in the stream)
    isA = core_of[src] < (NCORES // 2)
    a_cnt = np.bincount(dst[isA], minlength=N)
    b_cnt = np.bincount(dst[~isA], minlength=N)

    # node -> (tile, col): per core, LPT-style capacity-constrained
    # assignment balancing BOTH the A-side and B-side per-tile edge sums
    # (each lands ~1020 < 8*128, so per-tile-side chunk counts stay at the
    # flat ceiling); tiles then sorted by chunk need desc (aligns profiles
    # across cores) and renumbered.
    tile_of = np.full(N, -1, dtype=np.int64)
    col_of = np.full(N, -1, dtype=np.int64)
    ntiles_max = 0
    prov = []
    for c in range(NCORES):
        nodes = np.where(core_of == c)[0]
        nn = len(nodes)
        ntiles = -(-nn // P)
        ntiles_max = max(ntiles_max, ntiles)
        a = a_cnt[nodes]
        b = b_cnt[nodes]
        o2 = np.argsort(-(a + b), kind="stable")
        suma = np.zeros(ntiles)
        sumb = np.zeros(ntiles)
        cap = np.full(ntiles, P)
        tiles = [[] for _ in range(ntiles)]
        for i in o2:
            cand = np.where(cap > 0)[0]
            sc = np.maximum(suma[cand] + a[i], sumb[cand] + b[i]) + 1e-3 * (
                suma[cand] + sumb[cand]
            )
            t = cand[np.argmin(sc)]
            tiles[t].append(nodes[i])
            suma[t] += a[i]
            sumb[t] += b[i]
            cap[t] -= 1
        prov.append([np.array(t_, dtype=np.int64) for t_ in tiles])

    T = ntiles_max
    ca_t = np.zeros((NCORES, T), dtype=np.int64)
    cb_t = np.zeros((NCORES, T), dtype=np.int64)
    for c in range(NCORES):
        for t, nds in enumerate(prov[c]):
            ca_t[c, t] = -(-int(a_cnt[nds].sum()) // P)
            cb_t[c, t] = -(-int(b_cnt[nds].sum()) // P)
    CA = np.zeros(T, dtype=np.int64)
    CB = np.zeros(T, dtype=np.int64)
    for c in range(NCORES):
        perm = sorted(
            range(len(prov[c])), key=lambda t: -(ca_t[c, t] + cb_t[c, t])
        )
        for p_, t in enumerate(perm):
            nds = prov[c][t]
            tile_of[nds] = p_
            col_of[nds] = np.arange(len(nds))
            CA[p_] = max(CA[p_], ca_t[c, t])
            CB[p_] = max(CB[p_], cb_t[c, t])

    SLOTS = T * P
    HALFROWS = (NCORES // 2) * SLOTS
    assert HALFROWS <= 32768, HALFROWS
    slot_of = tile_of * P + col_of
    pos_of = core_of * SLOTS + slot_of

    ecore = core_of[dst]
    etile = tile_of[dst]
    eside = (~isA).astype(np.int64)
    esrcpos = pos_of[src]
    ecol = col_of[dst]

    G = -(-T // GSZ)
    groups = [list(range(g * GSZ, min((g + 1) * GSZ, T))) for g in range(G)]
    tot_chunks = int(np.sum(CA) + np.sum(CB))
    maxc_call = 0
    for g in groups:
        ca_g = int(sum(CA[p_] for p_ in g))
        cb_g = int(sum(CB[p_] for p_ in g))
        maxc_call = max(maxc_call, ca_g, cb_g)

    ekey = np.lexsort((esrcpos, etile, eside, ecore))
    es_core = ecore[ekey]
    es_side = eside[ekey]
    es_tile = etile[ekey]
    es_srcpos = esrcpos[ekey]
    es_col = ecol[ekey]
    keyv = (es_core * 2 + es_side) * T + es_tile
    uniq, starts = np.unique(keyv, return_index=True)
    ends = np.append(starts[1:], len(keyv))
    bnd = {int(u): (int(s0), int(e0)) for u, s0, e0 in zip(uniq, starts, ends)}

    idx_cores = []
    colidx_cores = []
    dinv_own_cores = []
    for c in range(NCORES):
        flat_idx = []
        flat_col = []
        dvo = np.zeros((P, T), dtype=np.float32)
        nds_c = np.where(core_of == c)[0]
        dvo[col_of[nds_c], tile_of[nds_c]] = dinv[nds_c]

        def emit(side, t, nchunks):
            k = (c * 2 + side) * T + t
            s0, e0 = bnd.get(k, (0, 0))
            sp = es_srcpos[s0:e0]
            cl = es_col[s0:e0]
            if side == 1:
                sp = sp - HALFROWS
            n_ = e0 - s0
            want = nchunks * P
            # pad lanes: idx 0 (any valid row) with col -1 -> sigma row is
            # all-zero, so the gathered garbage is multiplied by 0.
            ii = np.zeros(want, np.int64)
            cc = np.full(want, -1.0, np.float64)
            ii[:n_] = sp
            cc[:n_] = cl
            flat_idx.append(ii)
            flat_col.append(cc)

        for g in groups:
            for p_ in g:
                emit(0, p_, int(CA[p_]))
            for p_ in g:
                emit(1, p_, int(CB[p_]))
        fi = np.concatenate(flat_idx)
        fc = np.concatenate(flat_col)
        assert fi.size == tot_chunks * P
        assert fi.min() >= 0 and fi.max() < HALFROWS
        wrapped = fi.astype(np.int16).reshape(-1, 16).T.copy()
        idx_cores.append(np.tile(wrapped, (8, 1)))
        colidx_cores.append(fc.reshape(tot_chunks, P).T.astype(np.float16).copy())
        dinv_own_cores.append(dvo)

    iotaC = np.tile(
        np.arange(P, dtype=np.float16)[None, :], (P, maxc_call)
    ).reshape(P, maxc_call * P)

    return dict(
        T=T,
        SLOTS=SLOTS,
        CA=CA,
        CB=CB,
        groups=groups,
        tot_chunks=tot_chunks,
        maxc_call=maxc_call,
        core_of=core_of,
        slot_of=slot_of,
        pos_of=pos_of,
        dinv=dinv,
        idx_cores=idx_cores,
        colidx_cores=colidx_cores,
        dinv_own_cores=dinv_own_cores,
        iotaC=iotaC,
    )


# ----------------------------------------------------------------------------
# Device kernel
# ----------------------------------------------------------------------------
def _build(
    T,
    CA,
    CB,
    groups,
    tot_chunks,
    maxc_call,
    detect_races=True,
):
    import concourse.bass as bass
    import concourse.mybir as mybir
    import concourse.tile as tile
    from concourse import bacc

    f16 = mybir.dt.float16
    f32 = mybir.dt.float32
    i16 = mybir.dt.int16
    SLOTS = T * P
    ROWS = NCORES * SLOTS
    HALFROWS = ROWS // 2

    nc = bacc.Bacc(
        "TRN2",
        target_bir_lowering=False,
        num_devices=NCORES,
        num_swdge_queues=SWDGE_QUEUES,
        detect_race_conditions=detect_races,
    )
    qn = [0]

    def _next_q():
        qn[0] = (qn[0] + 1) % SWDGE_QUEUES
        return qn[0]

    dma_sems = [
        [nc.alloc_semaphore(f"swdge_dma_q{i}r{r}") for r in range(SEM_ROT)]
        for i in range(SWDGE_QUEUES)
    ]
    q_calls = [0] * SWDGE_QUEUES

    def _prep(out_ap, in_ap, idx_ap, n_idx, q):
        jq = q_calls[q]
        q_calls[q] += 1
        nc.gpsimd.dma_gather(
            out_ap,
            in_ap,
            idx_ap,
            n_idx,
            n_idx,
            P,
            prepare_only=True,
            sem=dma_sems[q][jq % SEM_ROT],
            queue_num=q,
        )

    xT_in = nc.dram_tensor("xT", [T, P, P], f16, kind="ExternalInput")
    w1_in = nc.dram_tensor("W1", [IN_CH, HID], f16, kind="ExternalInput")
    w2_in = nc.dram_tensor("W2", [HID, OUT_CH], f16, kind="ExternalInput")
    b1_in = nc.dram_tensor("b1bc", [P, HID], f32, kind="ExternalInput")
    b2_in = nc.dram_tensor("b2bc", [P, OUT_CH], f32, kind="ExternalInput")
    id_in = nc.dram_tensor("ident", [P, P], f16, kind="ExternalInput")
    col_in = nc.dram_tensor("colidx", [P, tot_chunks], f16, kind="ExternalInput")
    iota_in = nc.dram_tensor("iotaC", [P, maxc_call * P], f16, kind="ExternalInput")
    do_in = nc.dram_tensor("dinv_own", [P, T], f32, kind="ExternalInput")
    idx_in = nc.dram_tensor("idx", [P, tot_chunks * 8], i16, kind="ExternalInput")
    out_ext = nc.dram_tensor("out", [SLOTS, OUT_CH], f32, kind="ExternalOutput")

    with tile.TileContext(nc) as tc:
        with (
            tc.tile_pool(name="const", bufs=1) as cpool,
            tc.tile_pool(name="xt", bufs=3) as xtpool,
            tc.tile_pool(name="sig", bufs=4) as sigpool,
            tc.tile_pool(name="stg", bufs=STG_BUFS) as stgpool,
            tc.tile_pool(name="drain", bufs=3) as dpool,
            tc.tile_pool(name="psb", bufs=1, space="PSUM") as ps_build,
            tc.tile_pool(name="psa", bufs=4, space="PSUM") as ps_agg,
            tc.tile_pool(name="pst", bufs=2, space="PSUM") as ps_tr,
            tc.tile_pool(name="psm", bufs=1, space="PSUM") as ps_mm2,
            tc.tile_pool(name="dram", bufs=1, space="DRAM") as dram,
        ):
            # ---- constants into SBUF (idx first: it gates desc-gen) ----
            idx_sb = cpool.tile([P, tot_chunks * 8], i16)
            nc.sync.dma_start(out=idx_sb[:], in_=idx_in[:])
            w1_sb = cpool.tile([IN_CH, HID], f16)
            nc.sync.dma_start(out=w1_sb[:], in_=w1_in[:])
            w2_sb = cpool.tile([HID, OUT_CH], f16)
            nc.sync.dma_start(out=w2_sb[:], in_=w2_in[:])
            b1_sb = cpool.tile([P, HID], f32)
            nc.sync.dma_start(out=b1_sb[:], in_=b1_in[:])
            b2_sb = cpool.tile([P, OUT_CH], f32)
            nc.sync.dma_start(out=b2_sb[:], in_=b2_in[:])
            id_sb = cpool.tile([P, P], f16)
            nc.sync.dma_start(out=id_sb[:], in_=id_in[:])
            col_sb = cpool.tile([P, tot_chunks], f16)
            nc.sync.dma_start(out=col_sb[:], in_=col_in[:])
            iota_sb = cpool.tile([P, maxc_call * P], f16)
            nc.sync.dma_start(out=iota_sb[:], in_=iota_in[:])
            do_sb = cpool.tile([P, T], f32)
            nc.sync.dma_start(out=do_sb[:], in_=do_in[:])

            # own table rows stay in SBUF for the self-loop matmul
            own1_sb = cpool.tile([P, T * HID], f16)
            own2_sb = cpool.tile([P, T * OUT_CH], f16)

            own1 = dram.tile([SLOTS, HID], f16)
            table1 = dram.tile([ROWS, HID], f16, addr_space="Shared")
            shard2 = dram.tile([SLOTS, P], f16)
            table2 = dram.tile([ROWS, P], f16, addr_space="Shared")

            # ---- phase 1 (sharded): own1 = dinv_own * (x_own @ W1) ----
            for j0 in range(0, T, BB):
                nb = min(BB, T - j0)
                xt_t = xtpool.tile([P, nb * P], f16, tag="xt")
                nc.sync.dma_start(
                    out=xt_t[:].rearrange("p (t c) -> p t c", t=nb),
                    in_=xT_in[j0 : j0 + nb].rearrange("t p c -> p t c"),
                )
                for k in range(nb):
                    j = j0 + k
                    bps = ps_build.tile([P, HID], f32, tag="build")
                    nc.tensor.matmul(
                        bps[:],
                        lhsT=xt_t[:, k * P : (k + 1) * P],
                        rhs=w1_sb[:],
                        start=True,
                        stop=True,
                    )
                    nc.scalar.activation(
                        own1_sb[:, j * HID : (j + 1) * HID],
                        bps[:],
                        mybir.ActivationFunctionType.Copy,
                        scale=do_sb[:, j : j + 1],
                    )
                nc.sync.dma_start(
                    out=own1[j0 * P : (j0 + nb) * P, :].rearrange(
                        "(t p) f -> p t f", t=nb
                    ),
                    in_=own1_sb[:, j0 * HID : (j0 + nb) * HID].rearrange(
                        "p (t f) -> p t f", t=nb
                    ),
                )

            # ---- drains ----
            def drain(layer, p_, aps):
                dv = do_sb[:, p_ : p_ + 1]
                if layer == 0:
                    r1 = dpool.tile([P, HID], f32, tag="r1")
                    nc.scalar.activation(
                        r1[:], aps[:], mybir.ActivationFunctionType.Copy, scale=dv
                    )
                    nc.vector.tensor_add(r1[:], r1[:], b1_sb[:])
                    r3 = dpool.tile([P, HID], f16, tag="r3")
                    nc.scalar.activation(
                        r3[:], r1[:], mybir.ActivationFunctionType.Relu, scale=dv
                    )
                    psT = ps_tr.tile([P, P], f16, tag="tr")
                    nc.tensor.transpose(psT[:], r3[:], id_sb[:])
                    rT = dpool.tile([P, P], f16, tag="rT")
                    nc.scalar.activation(
                        rT[:], psT[:], mybir.ActivationFunctionType.Copy
                    )
                    ps2 = ps_mm2.tile([P, OUT_CH], f32, tag="mm2")
                    nc.tensor.matmul(
                        ps2[:], lhsT=rT[:], rhs=w2_sb[:], start=True, stop=True
                    )
                    nc.scalar.activation(
                        own2_sb[:, p_ * OUT_CH : (p_ + 1) * OUT_CH],
                        ps2[:],
                        mybir.ActivationFunctionType.Copy,
                    )
                    t2 = dpool.tile([P, P], f16, tag="t2")
                    nc.scalar.activation(
                        t2[:, 0:OUT_CH], ps2[:], mybir.ActivationFunctionType.Copy
                    )
                    nc.vector.memset(t2[:, OUT_CH:P], 0.0)
                    nc.sync.dma_start(
                        out=shard2[p_ * P : (p_ + 1) * P, :], in_=t2[:]
                    )
                else:
                    o1 = dpool.tile([P, OUT_CH], f32, tag="o1")
                    nc.scalar.activation(
                        o1[:], aps[:], mybir.ActivationFunctionType.Copy, scale=dv
                    )
                    nc.vector.tensor_add(o1[:], o1[:], b2_sb[:])
                    nc.sync.dma_start(
                        out=out_ext[p_ * P : (p_ + 1) * P, :], in_=o1[:]
                    )

            # ---- aggregation ----
            def aggregate(layer):
                tab = table1 if layer == 0 else table2
                nfeat = HID if layer == 0 else OUT_CH
                own_sb = own1_sb if layer == 0 else own2_sb
                ownw = HID if layer == 0 else OUT_CH

                sides = []
                coff = 0
                for gi, g in enumerate(groups):
                    for side in (0, 1):
                        cnt = CA if side == 0 else CB
                        c_g = int(sum(int(cnt[p_]) for p_ in g))
                        sides.append((gi, side, coff, c_g))
                        coff += c_g
                staged = {}

                def stage(i):
                    gi, side, soff, c_g = sides[i]
                    if c_g == 0:
                        staged[i] = (None, None, [])
                        return
                    if side == 0:
                        stA = stgpool.tile([P, maxc_call, P], f16, tag="stgA")
                        st = stA
                    else:
                        stB = stgpool.tile([P, maxc_call, P], f16, tag="stgB")
                        st = stB
                    percall = []
                    for s_ in range(0, c_g, CALL_CAP):
                        n_ = min(CALL_CAP, c_g - s_)
                        q = _next_q()
                        _prep(
                            st[:, s_ : s_ + n_, :],
                            tab[0:HALFROWS, :]
                            if side == 0
                            else tab[HALFROWS:ROWS, :],
                            idx_sb[:, (soff + s_) * 8 : (soff + s_ + n_) * 8],
                            n_ * P,
                            q,
                        )
                        percall.append(q)
                    cnts = {}
                    for q in percall:
                        cnts[q] = cnts.get(q, 0) + 1
                    if side == 0:
                        sgA = sigpool.tile([P, maxc_call * P], f16, tag="sgA")
                        sg = sgA
                    else:
                        sgB = sigpool.tile([P, maxc_call * P], f16, tag="sgB")
                        sg = sgB
                    nc.vector.tensor_tensor(
                        sg[:, : c_g * P].rearrange("p (k c) -> p k c", k=c_g),
                        iota_sb[:, : c_g * P].rearrange("p (k c) -> p k c", k=c_g),
                        col_sb[:, soff : soff + c_g]
                        .unsqueeze(-1)
                        .broadcast_to([P, c_g, P]),
                        mybir.AluOpType.is_equal,
                    )
                    staged[i] = (st, sg, sorted(cnts.items()))

                def fire(i):
                    for q, k in staged[i][2]:
                        nc.gpsimd.trigger_dma(count=k, queue_num=q)

                def consume(gi):
                    stA, sgA_, _ = staged[2 * gi]
                    stB, sgB_, _ = staged[2 * gi + 1]
                    a_off = 0
                    b_off = 0
                    for p_ in groups[gi]:
                        aps = ps_agg.tile([P, nfeat], f32, tag="agg")
                        k = 0
                        for ci in range(int(CA[p_])):
                            cc = a_off + ci
                            nc.tensor.matmul(
                                aps[:],
                                lhsT=sgA_[:, cc * P : (cc + 1) * P],
                                rhs=stA[:, cc, 0:nfeat],
                                start=(k == 0),
                                stop=False,
                            )
                            k += 1
                        for ci in range(int(CB[p_])):
                            cc = b_off + ci
                            nc.tensor.matmul(
                                aps[:],
                                lhsT=sgB_[:, cc * P : (cc + 1) * P],
                                rhs=stB[:, cc, 0:nfeat],
                                start=(k == 0),
                                stop=False,
                            )
                            k += 1
                        # self-loop row: aps += I @ own_row
                        nc.tensor.matmul(
                            aps[:],
                            lhsT=id_sb[:],
                            rhs=own_sb[:, p_ * ownw : (p_ + 1) * ownw],
                            start=(k == 0),
                            stop=True,
                        )
                        a_off += int(CA[p_])
                        b_off += int(CB[p_])
                        drain(layer, p_, aps)

                # Emission is NOT pipelined: each trigger immediately follows
                # its side's preps so Tile's no_sync prep->trigger links bind
                # exactly (a trigger with an empty pending list free-floats
                # and gets hoisted by the scheduler). The desc-gen pipelining
                # is done post-compile by _reorder_pool.
                for i in range(len(sides)):
                    stage(i)
                    fire(i)
                    if i % 2 == 1:
                        consume(i // 2)

            def cc1():
                nc.gpsimd.collective_compute(
                    "AllGather",
                    mybir.AluOpType.bypass,
                    replica_groups=[list(range(NCORES))],
                    ins=[own1.opt()],
                    outs=[table1.opt()],
                )

            def cc2():
                nc.gpsimd.collective_compute(
                    "AllGather",
                    mybir.AluOpType.bypass,
                    replica_groups=[list(range(NCORES))],
                    ins=[shard2.opt()],
                    outs=[table2.opt()],
                )

            cc1()
            aggregate(0)
            cc2()
            aggregate(1)

    nc.compile()  # bacc passes: library loads, register allocation, DCE
    _strip_tile_sync(nc, mybir)
    _reorder_pool(nc, mybir)
    _fix_swdge_sems(nc, mybir)
    _split_sync_waits(nc, mybir, max_waits=1)
    return nc


def _reorder_pool(nc, mybir):
    """Software-pipeline desc-gen by permuting the Pool subsequence.

    The scheduled stream has each gather prep just before its trigger. Hoist
    prep units ([RegisterMove, DMAGatherAnt] pairs) earlier among the Pool
    instructions — preserving prep relative order (ring FIFO order and Pool
    tick counts depend on it) and the relative order of everything else —
    so desc-gen runs ahead of the trigger/consume stages, bounded by LEAD_Q
    untriggered calls per queue. At most CC1_LEAD units go ahead of the
    first AllGather so its issue (which blocks the in-order Pool stream on
    the phase-1 writes) isn't pushed behind ~100us of desc-gen.
    """
    for fn in nc.m.functions:
        for bb in fn.blocks:
            insts = bb.instructions
            pool_pos = [
                i
                for i, ins in enumerate(insts)
                if getattr(ins, "engine", None) == mybir.EngineType.Pool
            ]
            if not pool_pos:
                continue
            pool = [insts[i] for i in pool_pos]
            # Pair each prep with the RegisterMove writing its num_idxs reg
            # (regrefs are single-assignment, one write per prep; the pair
            # may be separated by EventSemaphores in the scheduled stream).
            import re as _re

            def _regref(ap_str):
                m = _re.search(r"regref='([^']+)'", ap_str)
                return m.group(1) if m else None

            reg_writer = {}
            for ins in pool:
                if type(ins).__name__ == "InstRegisterMove" and ins.outs:
                    rr = _regref(str(ins.outs[0]))
                    if rr is not None:
                        assert rr not in reg_writer, rr
                        reg_writer[rr] = ins
            consumed = set()
            units = []  # (queue, [regmove, prep])
            others = []
            for ins in pool:
                tn = type(ins).__name__
                if tn == "InstDMAGatherAnt" and getattr(ins, "gen_mode", 0) == 1:
                    rr = _regref(str(ins.ins[-1]))
                    rm = reg_writer.get(rr)
                    assert rm is not None, rr
                    consumed.add(id(rm))
                    units.append((ins.queue_num, [rm, ins]))
            for ins in pool:
                tn = type(ins).__name__
                if tn == "InstDMAGatherAnt" and getattr(ins, "gen_mode", 0) == 1:
                    continue
                if id(ins) in consumed:
                    continue
                others.append(ins)
            if not units:
                continue
            # The SWDGE ucode library must be loaded before ANY gpsimd
            # library op (gather desc-gen, collective): hoist the reload to
            # the front of this block's Pool stream.
            relo = [
                o
                for o in others
                if type(o).__name__ == "InstPseudoReloadLibraryIndex"
            ]
            if relo:
                others.remove(relo[0])
                others.insert(0, relo[0])
            new_pool = []
            inflight = {}
            emitted = {}
            fired = {}
            ucur = 0
            ncc = 0
            # no desc-gen before the SWDGE ucode library reload
            lib_done = not any(
                type(o).__name__ == "InstPseudoReloadLibraryIndex" for o in others
            )

            def can_emit():
                if ucur >= len(units):
                    return False
                q = units[ucur][0]
                return inflight.get(q, 0) < LEAD_Q

            def emit_unit():
                nonlocal ucur
                q, ins_list = units[ucur]
                new_pool.extend(ins_list)
                inflight[q] = inflight.get(q, 0) + 1
                emitted[q] = emitted.get(q, 0) + 1
                ucur += 1

            for ins in others:
                tn = type(ins).__name__
                if not lib_done:
                    new_pool.append(ins)
                    if tn == "InstPseudoReloadLibraryIndex":
                        lib_done = True
                    continue
                if tn == "InstTriggerDma":
                    q = ins.queue_num
                    k = getattr(ins, "_count", None) or 0
                    # force this trigger's calls into the stream first
                    while emitted.get(q, 0) < fired.get(q, 0) + k:
                        assert ucur < len(units)
                        emit_unit()
                    fired[q] = fired.get(q, 0) + k
                    inflight[q] = emitted.get(q, 0) - fired.get(q, 0)
                # greedy hoist, capped before the first collective
                while can_emit() and not (ncc == 0 and ucur >= CC1_LEAD):
                    emit_unit()
                new_pool.append(ins)
                if tn == "InstCollectiveCompute":
                    ncc += 1
            while ucur < len(units):
                emit_unit()
            assert len(new_pool) == len(pool)
            for pos, ins in zip(pool_pos, new_pool):
                insts[pos] = ins
            bb.instructions = insts


def _add_wait(ins, mybir, sid, snm, val):
    si = ins.sync_info
    if si is None:
        si = mybir.SyncInfo(on_wait=[], on_update=[])
    si.on_wait = list(si.on_wait or []) + [
        mybir.SyncWait(
            sync_type="semaphore",
            id=sid,
            ant_name=snm,
            wait_mode="sem-ge-imm",
            wait_value=val,
            wait_reg=None,
        )
    ]
    ins.sync_info = si


def _strip_tile_sync(nc, mybir):
    """Delete IncSwdgeSem pre-bumps + every DMASW lane-sem wait; clear all
    waits from gather preps (re-adding an idx-load gate on the first one);
    clear data waits from Pool-stream gate instructions (EventSemaphore /
    NoOp) up to the last trigger so desc-gen free-runs."""
    streams = []
    for fn in nc.m.functions:
        for bb in fn.blocks:
            streams.append(bb)
    # DMAHW completion gates: cumulative per-lane counts at the idx load and
    # at each write into the collectives' source tensors (Tile's own gates
    # for these were hoisted onto Pool EventSemaphores, which get stripped).
    idx_gate = None
    cum = {}
    sem_ids = {}
    thr = {"own1": {}, "shard2": {}}
    for bb in streams:
        for ins in bb.instructions:
            if type(ins).__name__ != "InstDMACopy":
                continue
            si = ins.sync_info
            for u in (si.on_update or []) if si else []:
                nm = u.ant_name or ""
                if nm.startswith("DMAHW"):
                    cum[nm] = cum.get(nm, 0) + (u.update_value or 16)
                    sem_ids[nm] = u.id
                    outs = str(ins.outs[0])
                    if idx_gate is None and "idx_sb" in outs:
                        idx_gate = (u.id, nm, cum[nm])
                    for t in ("own1", "shard2"):
                        if f"{t}_" in outs or f"'{t}'" in outs:
                            thr[t][nm] = cum[nm]

    # last trigger position per bb for the Pool-gate strip range
    last_trig = {}
    for bi, bb in enumerate(streams):
        for pos, ins in enumerate(bb.instructions):
            if type(ins).__name__ == "InstTriggerDma":
                last_trig[bi] = pos

    first_prep = None
    n_inc = 0
    for bi, bb in enumerate(streams):
        out = []
        for pos, ins in enumerate(bb.instructions):
            tn = type(ins).__name__
            if tn == "InstIncSwdgeSem":
                n_inc += 1
                continue
            si = ins.sync_info
            if si and si.on_wait:
                w2 = [
                    w
                    for w in si.on_wait
                    if not (w.ant_name or "").startswith("DMASW")
                ]
                if tn == "InstDMAGatherAnt" and getattr(ins, "gen_mode", 0) == 1:
                    w2 = []
                elif (
                    ins.engine == mybir.EngineType.Pool
                    and tn in ("InstEventSemaphore", "InstNoOp")
                    and pos <= last_trig.get(bi, -1)
                ):
                    w2 = [
                        w
                        for w in w2
                        if (w.ant_name or "").startswith(
                            ("barrier_", "swdge_dma_")
                        )
                    ]
                si.on_wait = w2
            if (
                tn == "InstDMAGatherAnt"
                and getattr(ins, "gen_mode", 0) == 1
                and first_prep is None
            ):
                first_prep = ins
            out.append(ins)
        bb.instructions = out
    assert n_inc > 0
    assert first_prep is not None
    if idx_gate is not None:
        _add_wait(first_prep, mybir, idx_gate[0], idx_gate[1], idx_gate[2])

    # re-gate the collectives on their source tensors' write completion
    assert thr["own1"] and thr["shard2"], thr
    n_cc = 0
    for bb in streams:
        for ins in bb.instructions:
            if type(ins).__name__ != "InstCollectiveCompute":
                continue
            t = "own1" if n_cc == 0 else "shard2"
            n_cc += 1
            have = {
                (w.ant_name, w.wait_value) for w in (ins.sync_info.on_wait or [])
            }
            for nm, v in sorted(thr[t].items()):
                if (nm, v) not in have:
                    _add_wait(ins, mybir, sem_ids[nm], nm, v)
    assert n_cc == 2, n_cc


def _fix_swdge_sems(nc, mybir):
    """Sem rewiring for the stripped, reordered prep/trigger SWDGE path.

    1. Data RAW: the first matmul reading each staging-tile instance waits
       on every covering gather call: sem_q(rot) >= 16*(call# in rot + 1).
    2. WAR + collective gating on triggers: a trigger firing a DMA that
       overwrites a staging slot waits on PE progress (last reader of the
       instance STG_BUFS allocations back); the first trigger per queue
       after each AllGather waits on its completion.
    3. Ring throttle: prep #j on queue q waits sem_q so at most PREP_DEPTH
       calls per queue are in flight (untriggered ring-entry cap).
    """
    import re

    streams = []
    for fn in nc.m.functions:
        for bb in fn.blocks:
            streams.append(bb)

    queue_sems = {}
    pe_sem = None
    cc_sem = None
    for bb in streams:
        for ins in bb.instructions:
            si = ins.sync_info
            if not si:
                continue
            for u in si.on_update or []:
                nm = u.ant_name or ""
                if nm.startswith("swdge_dma_q"):
                    qs, rs = nm[11:].split("r")
                    queue_sems[(int(qs), int(rs))] = (u.id, nm)
            for w in si.on_wait or []:
                nm = w.ant_name or ""
                if nm.startswith("PE_") and pe_sem is None:
                    pe_sem = (w.id, nm)
                if nm.startswith("Collectives") and cc_sem is None:
                    cc_sem = (w.id, nm)
            if type(ins).__name__ == "InstCollectiveCompute" and cc_sem is None:
                for u in si.on_update or []:
                    if (u.ant_name or "").startswith("Collectives"):
                        cc_sem = (u.id, u.ant_name)
    assert pe_sem is not None and cc_sem is not None, (pe_sem, cc_sem)

    pat = re.compile(r"\b(st[AB])_(\d+)\b")

    def _stg_name(ap):
        m = pat.search(str(ap))
        return m.group(0) if m else None

    # pass 1 prep: per-queue call indices; staging instances: creation order
    # (per tag), covering calls, first/last matmul readers
    inst_order = {"stA": [], "stB": []}
    seen = set()
    first_reader = {}
    last_reader_n = {}
    inst_calls = {}
    prep_info = {}
    q_count = {}
    pe_n = 0
    for bb in streams:
        for ins in bb.instructions:
            tn = type(ins).__name__
            if tn == "InstMatmult":
                pe_n += 1
                for ap in ins.ins or []:
                    nm = _stg_name(ap)
                    if nm:
                        if nm not in first_reader:
                            first_reader[nm] = ins
                        last_reader_n[nm] = pe_n
            elif tn == "InstDMAGatherAnt" and getattr(ins, "gen_mode", 0) == 1:
                q = ins.queue_num
                jq = q_count.get(q, 0)
                q_count[q] = jq + 1
                nm = _stg_name(ins.outs[0])
                prep_info[ins.name] = (q, jq, nm)
                if nm:
                    inst_calls.setdefault(nm, []).append((q, jq))
                    if nm not in seen:
                        seen.add(nm)
                        inst_order[nm[:3]].append(nm)
    assert first_reader, "staging-name regex matched nothing"
    prev_inst = {}
    for tag, lst in inst_order.items():
        for i, nm in enumerate(lst):
            if i >= STG_BUFS:
                prev_inst[nm] = lst[i - STG_BUFS]

    # 1. data RAW waits on first readers (per (queue, rotation) max target)
    for nm, rd in first_reader.items():
        per_qr = {}
        for q, jq in inst_calls.get(nm, []):
            k = (q, jq % SEM_ROT)
            per_qr[k] = max(per_qr.get(k, -1), jq // SEM_ROT)
        for k, t in sorted(per_qr.items()):
            if k in queue_sems:
                sid, snm = queue_sems[k]
                _add_wait(rd, mybir, sid, snm, 16 * (t + 1))

    # 2 + 3. WAR + collective gating on triggers, throttle on preps
    pending_prep = {}
    cc_count = 0
    cc_pending = set()
    for bb in streams:
        for ins in bb.instructions:
            tn = type(ins).__name__
            if tn == "InstCollectiveCompute":
                cc_count += 1
                cc_pending = set(range(SWDGE_QUEUES))
            elif tn == "InstDMAGatherAnt" and getattr(ins, "gen_mode", 0) == 1:
                q, jq, nm = prep_info[ins.name]
                pending_prep.setdefault(q, []).append(ins.name)
                if jq >= PREP_DEPTH:
                    jt = jq - PREP_DEPTH
                    k = (q, jt % SEM_ROT)
                    if k in queue_sems:
                        sid, snm = queue_sems[k]
                        _add_wait(ins, mybir, sid, snm, 16 * (jt // SEM_ROT + 1))
            elif tn == "InstTriggerDma":
                if ins.queue_num in cc_pending and cc_sem is not None:
                    _add_wait(ins, mybir, cc_sem[0], cc_sem[1], cc_count)
                    cc_pending.discard(ins.queue_num)
                k = getattr(ins, "_count", None)
                lst = pending_prep.get(ins.queue_num, [])
                pns = lst[:k] if k else lst
                pending_prep[ins.queue_num] = lst[len(pns) :]
                if not pns or pe_sem is None:
                    continue
                tgt = 0
                for pn in pns:
                    nm = prep_info[pn][2]
                    prev = prev_inst.get(nm) if nm else None
                    if prev:
                        tgt = max(tgt, last_reader_n.get(prev, 0))
                if tgt > 0:
                    sid, snm = pe_sem
                    _add_wait(ins, mybir, sid, snm, tgt)


def _split_sync_waits(nc, mybir, max_waits=1):
    """This walrus build rejects instructions with more than `max_waits` sync
    waits; hoist excess waits onto injected same-engine InstNoOps."""
    n_split = 0
    for fn in nc.m.functions:
        for bb in fn.blocks:
            out = []
            changed = False
            for ins in bb.instructions:
                si = ins.sync_info
                if si is not None and si.on_wait and len(si.on_wait) > max_waits:
                    waits = list(si.on_wait)
                    excess = waits[:-max_waits]
                    for i in range(0, len(excess), max_waits):
                        nop = mybir.InstNoOp(
                            name=nc.get_next_instruction_name(),
                            sync_info=mybir.SyncInfo(
                                on_wait=excess[i : i + max_waits], on_update=[]
                            ),
                            bass_nofuse=True,
                            engine=ins.engine,
                        )
                        out.append(nop)
                        n_split += 1
                    si.on_wait = waits[-max_waits:]
                    ins.sync_info = si
                    changed = True
                out.append(ins)
            if changed:
                bb.instructions = out
    return n_split


# ----------------------------------------------------------------------------
# Entry point
# ----------------------------------------------------------------------------
def kernel(x, edge_index, W1, b1, W2, b2):
    global LAST_RESULTS
    from concourse.bass_utils import run_bass_kernel_spmd

    x = np.asarray(x)
    W1a = np.asarray(W1)
    b1a = np.asarray(b1)
    W2a = np.asarray(W2)
    b2a = np.asarray(b2)

    key = hash(np.asarray(edge_index)[:, :: E // 997].tobytes())
    if key not in _CACHE:
        plan = _plan(edge_index)
        nc = _build(
            plan["T"],
            plan["CA"],
            plan["CB"],
            plan["groups"],
            plan["tot_chunks"],
            plan["maxc_call"],
        )
        _CACHE[key] = (plan, nc)
    plan, nc = _CACHE[key]

    T = plan["T"]
    SLOTS = plan["SLOTS"]

    core_of = plan["core_of"]
    slot_of = plan["slot_of"]

    in_common = {
        "W1": W1a.astype(np.float16),
        "W2": W2a.astype(np.float16),
        "b1bc": np.broadcast_to(b1a.astype(np.float32), (P, HID)).copy(),
        "b2bc": np.broadcast_to(b2a.astype(np.float32), (P, OUT_CH)).copy(),
        "ident": np.eye(P, dtype=np.float16),
        "iotaC": plan["iotaC"],
    }
    in_maps = []
    xf16 = x.astype(np.float16)
    for c in range(NCORES):
        sel = core_of == c
        nodes = np.where(sel)[0]
        xTflat = np.zeros((P, SLOTS), dtype=np.float16)
        xTflat[:, slot_of[nodes]] = xf16[nodes].T
        xTc = xTflat.reshape(P, T, P).transpose(1, 0, 2).copy()
        m = dict(in_common)
        m["xT"] = xTc
        m["colidx"] = plan["colidx_cores"][c]
        m["dinv_own"] = plan["dinv_own_cores"][c]
        m["idx"] = plan["idx_cores"][c]
        in_maps.append(m)

    res = run_bass_kernel_spmd(nc, in_maps, core_ids=list(range(NCORES)))
    LAST_RESULTS = res

    out = np.empty((N, OUT_CH), dtype=np.float32)
    for c in range(NCORES):
        sel = core_of == c
        out[sel] = res.results[c]["out"][slot_of[sel]]
    return out
